# revision 1
# baseline (speedup 1.0000x reference)
"""GATv2 edge predictor on 8 TRN2 NeuronCores.

Sharding: edges partitioned by dst node range (6272 nodes/core); each core
aggregates its own nodes' messages fully locally (segment-softmax without
max-subtraction, exp-weighted scatter-add via one-hot matmul into PSUM),
then AllGathers the per-node projections needed by the next phase.
"""
import math
import numpy as np

import concourse.bass as bass
import concourse.tile as tile
import concourse.mybir as mybir
from concourse.bass_utils import run_bass_kernel_spmd

# ---------------------------------------------------------------- wait patch
# This container's walrus build rejects >1 sync-wait command per instruction.
# Hoist extra waits onto single-wait NoOps on the same engine (engine streams
# execute in order, so gating is equivalent), and split the kernel-tail
# drain's per-proc waits the same way.
import bass_rust
from concourse.vector_clock import ScopedClock
from bass_rust import VectorClock as _RVC

_orig_commit = tile.TileContext._commit_and_lower
_ctr = [0]


def _split_commit(self, inst, original_block, old_bb_map, bb_to_exit_bb):
    si = getattr(inst, "sync_info", None)
    if si is not None:
        waits = list(si.on_wait)
        if len(waits) > 1:
            hoist = [w for w in waits if w.wait_reg is None]
            keep = [w for w in waits if w.wait_reg is not None]
            if not keep:
                keep = [hoist.pop()]
            for w in hoist:
                _ctr[0] += 1
                n = mybir.InstNoOp(name=f"TW-{_ctr[0]}", ins=[], outs=[])
                n.engine = inst.engine
                n.sync_info = bass_rust.SyncInfo(on_wait=[w], on_update=[])
                _orig_commit(self, n, original_block, old_bb_map, bb_to_exit_bb)
            inst.sync_info = bass_rust.SyncInfo(
                on_wait=keep, on_update=list(si.on_update)
            )
    return _orig_commit(self, inst, original_block, old_bb_map, bb_to_exit_bb)


def _patched_drain_and_barrier(self, tick_clock, wait_clock):
    ticks = list(tick_clock.global_clock)
    for i, t in enumerate(ticks):
        if t > 0:
            sub = [t if j == i else 0 for j in range(len(ticks))]
            nop_inst = self.nc.sync.nop(nofuse=True).ins
            wait_clock.add_sem_waits(nop_inst, ScopedClock({None: _RVC(sub)}))
    self.nc.sync.drain()
    self.nc.all_engine_barrier()
    assert self.sems is not None
    popped = self.nc._tile_sem_poison_stack.pop()
    assert popped is self._sem_poison
    self.nc.clear_and_free_semaphores(list(self.sems.allocated().values()))
    self.nc.all_engine_barrier()


tile.TileContext._commit_and_lower = _split_commit
tile.TileContext._drain_and_barrier = _patched_drain_and_barrier
# ------------------------------------------------------------ end wait patch

F32 = mybir.dt.float32
I32 = mybir.dt.int32
AF = mybir.ActivationFunctionType
OP = mybir.AluOpType
AX = mybir.AxisListType

NC_CORES = 8
P = 128
IN = 128
H = 2
C = 64
HC = H * C          # 128
OUT = 64
ED = 16
NEG = 0.2


_TRACE = [False]
_LAST = [None]


def _ceil_to(x, m):
    return ((x + m - 1) // m) * m


def host_prep(x, edge_index, edge_attr, n_nodes):
    """Structure-only sharding prep. Returns per-core index/attr arrays."""
    E = edge_index.shape[1]
    src = edge_index[0].astype(np.int64)
    dst = edge_index[1].astype(np.int64)

    n_pad = _ceil_to(n_nodes, NC_CORES * P)          # 50176
    nodes_per_core = n_pad // NC_CORES               # 6272
    nwin = nodes_per_core // P                       # 49

    order = np.argsort(dst, kind="stable")
    dsts = dst[order]

    # window boundaries over the sorted edge list (global windows 0..8*nwin-1)
    tot_win = NC_CORES * nwin
    wbound = np.searchsorted(dsts, np.arange(tot_win + 1) * P)
    wcnt = np.diff(wbound)
    M = max(1, int(np.max((wcnt + P - 1) // P)))

    cores = []
    for cidx in range(NC_CORES):
        base = cidx * nodes_per_core
        srcg = np.zeros((nwin, M, P), np.int32)
        dstsh = np.zeros((nwin, M, P), np.int32)
        dstwin = np.full((nwin, M, P), -1.0, np.float32)
        attr_rows = np.zeros((nwin, P, M * (ED + 1)), np.float32)
        attr_rows.reshape(nwin, P, M, ED + 1)[:, :, :, ED] = 1.0
        attrT = np.zeros((nwin, ED, M * P), np.float32)
        outmap = np.full((nwin, M, P), -1, np.int64)
        for w in range(nwin):
            gw = cidx * nwin + w
            e0, e1 = wbound[gw], wbound[gw + 1]
            cnt = e1 - e0
            if cnt == 0:
                continue
            eids = order[e0:e1]
            j = np.arange(cnt) // P
            p = np.arange(cnt) % P
            srcg[w, j, p] = src[eids]
            dstsh[w, j, p] = dst[eids] - base
            dstwin[w, j, p] = (dst[eids] - base - w * P).astype(np.float32)
            a = edge_attr[eids]                      # [cnt, ED]
            attr_rows[w, p[:, None],
                      (j * (ED + 1))[:, None] + np.arange(ED)[None, :]] = a
            attrT[w][:, j * P + p] = a.T
            outmap[w, j, p] = eids
        oneh = (dstwin[..., None] == np.arange(P, dtype=np.float32)).astype(np.float32)
        # oneh: [nwin, M, P(edge lane), P(node)]
        S_d = np.ascontiguousarray(
            oneh.transpose(0, 2, 1, 3).reshape(nwin, P, M * P))
        St_d = np.ascontiguousarray(
            oneh.transpose(0, 3, 1, 2).reshape(nwin, P, M * P))
        cores.append(
            dict(
                S_d=S_d, St_d=St_d,
                srcg=np.ascontiguousarray(srcg.transpose(2, 0, 1).reshape(P, nwin * M)),
                dstsh=np.ascontiguousarray(dstsh.transpose(2, 0, 1).reshape(P, nwin * M)),
                dstwin=np.ascontiguousarray(dstwin.transpose(2, 0, 1).reshape(P, nwin * M)),
                attr_rows=attr_rows,
                attrT=attrT,
                outmap=outmap,
            )
        )
    return cores, n_pad, nodes_per_core, nwin, M


def build_nc(nwin, M, n_pad, npc, nocoll=False):
    """Build the SPMD Bass program (identical for all cores)."""
    nc = bass.Bass()
    NT = nwin * M

    def param(name, shape, dt=F32):
        return nc.declare_dram_parameter(name, list(shape), dt, isOutput=False)

    xT = param("xT", [IN, npc])                      # this core's x shard, transposed
    attr_rows = param("attr_rows", [nwin, P, M * (ED + 1)])
    attrT = param("attrT", [nwin, ED, M * P])
    srcg = param("srcg", [P, NT], I32)
    dstsh = param("dstsh", [P, NT], I32)
    S_d = param("S_d", [nwin, P, M * P])
    St_d = param("St_d", [nwin, P, M * P])
    W1cat = param("W1cat", [IN, 2 * HC])
    b1cat = param("b1cat", [P, 2 * HC])
    We1 = param("We1", [ED, HC])
    att1 = param("att1", [P, HC])
    gb1 = param("gb1", [P, HC])
    W2cat = param("W2cat", [HC, 2 * OUT])
    b2cat = param("b2cat", [P, 2 * OUT])
    We2 = param("We2", [ED, OUT])
    att2 = param("att2", [P, OUT])
    gb2 = param("gb2", [P, OUT])
    WmCat = param("WmCat", [OUT, 2 * OUT])
    bm1 = param("bm1", [P, OUT])
    wm2 = param("wm2", [P, OUT])
    bm2 = param("bm2", [P, 1])
    ident = param("ident", [P, P])
    outv = nc.declare_dram_parameter("outv", [P, NT], F32, isOutput=True)

    with tile.TileContext(nc) as tc:
        with (
            tc.tile_pool(name="const", bufs=1) as cp,
            tc.tile_pool(name="bulk", bufs=1) as bulk,
            tc.tile_pool(name="sb", bufs=4) as sb,
            tc.tile_pool(name="gat", bufs=10) as gat,
            tc.tile_pool(name="sw", bufs=2) as swp,
            tc.tile_pool(name="ps", bufs=2, space="PSUM") as ps,
            tc.tile_pool(name="pg", bufs=1, space="PSUM") as pg,
            tc.tile_pool(name="psea", bufs=5, space="PSUM") as psea,
            tc.tile_pool(name="dram", bufs=1, space="DRAM") as dram,
        ):
            # ---- constants to SBUF
            def ld(ap, shape, dt=F32):
                t = cp.tile(list(shape), dt, tag=f"c{ld.i}")
                ld.i += 1
                nc.sync.dma_start(out=t[:], in_=ap[:])
                return t
            ld.i = 0

            tW1 = ld(W1cat, [IN, 2 * HC])
            tb1c = ld(b1cat, [P, 2 * HC])
            tWe1 = ld(We1, [ED, HC])
            tatt1 = ld(att1, [P, HC])
            tgb1 = ld(gb1, [P, HC])
            tW2 = ld(W2cat, [HC, 2 * OUT])
            tb2c = ld(b2cat, [P, 2 * OUT])
            tWe2 = ld(We2, [ED, OUT])
            tatt2 = ld(att2, [P, OUT])
            tgb2 = ld(gb2, [P, OUT])
            tWm = ld(WmCat, [OUT, 2 * OUT])
            tbm1 = ld(bm1, [P, OUT])
            twm2 = ld(wm2, [P, OUT])
            tbm2 = ld(bm2, [P, 1])
            tident = ld(ident, [P, P])
            tones = cp.tile([P, 1], F32, tag="ones")
            nc.vector.memset(tones[:], 1.0)

            txT = bulk.tile([IN, npc], F32, tag="xT")
            nc.sync.dma_start(out=txT[:], in_=xT[:])
            tsrc = bulk.tile([P, NT], I32, tag="srcg")
            nc.sync.dma_start(out=tsrc[:], in_=srcg[:])
            tdsh = bulk.tile([P, NT], I32, tag="dstsh")
            nc.sync.dma_start(out=tdsh[:], in_=dstsh[:])
            tlatT = bulk.tile([ED, nwin * P], F32, tag="latT")

            # internal DRAM
            xl_sh = dram.tile([npc, HC], F32, tag="xl_sh")
            xr_sh = dram.tile([npc, HC], F32, tag="xr_sh")
            xl_full = dram.tile([n_pad, HC], F32, tag="xl_full")
            xl2_sh = dram.tile([npc, OUT], F32, tag="xl2_sh")
            xr2_sh = dram.tile([npc, OUT], F32, tag="xr2_sh")
            xl2_full = dram.tile([n_pad, OUT], F32, tag="xl2_full")
            u_sh = dram.tile([npc, OUT], F32, tag="u_sh")
            v_sh = dram.tile([npc, OUT], F32, tag="v_sh")
            u_full = dram.tile([n_pad, OUT], F32, tag="u_full")

            # ================= Phase 0: xl/xr projections for the shard
            for w in range(nwin):
                pm = ps.tile([P, 2 * HC], F32, tag="pagg", space="PSUM")
                nc.tensor.matmul(
                    out=pm[:], lhsT=txT[:, w * P:(w + 1) * P], rhs=tW1[:],
                    start=True, stop=True,
                )
                so = sb.tile([P, 2 * HC], F32, tag="p0o")
                nc.vector.tensor_add(out=so[:], in0=pm[:], in1=tb1c[:])
                nc.sync.dma_start(out=xl_sh[w * P:(w + 1) * P, :], in_=so[:, :HC])
                nc.sync.dma_start(out=xr_sh[w * P:(w + 1) * P, :], in_=so[:, HC:])
            def allgather(shard, full):
                if nocoll:
                    nc.sync.dma_start(out=full[0:npc, :], in_=shard[:])
                else:
                    nc.gpsimd.collective_compute(
                        "AllGather", OP.bypass,
                        replica_groups=[list(range(NC_CORES))],
                        ins=[shard.opt()], outs=[full.opt()],
                    )

            allgather(xl_sh, xl_full)

            # ============ Phase 2 & 3: the two GATv2 layers
            def gat_layer(layer):
                if layer == 1:
                    heads, ch, hc = H, C, HC
                    full, shard_r = xl_full, xr_sh
                    tWe, tatt, tgb = tWe1, tatt1, tgb1
                    loc_l = xl_sh
                else:
                    heads, ch, hc = 1, OUT, OUT
                    full, shard_r = xl2_full, xr2_sh
                    tWe, tatt, tgb = tWe2, tatt2, tgb2
                    loc_l = xl2_sh
                ew = hc + heads + (ED + 1 if layer == 1 else 0)
                for w in range(nwin):
                    pagg = ps.tile([P, ew], F32, tag="pagg",
                                   space="PSUM")
                    taT = swp.tile([ED, M * P], F32, tag="aT")
                    nc.sync.dma_start(out=taT[:], in_=attrT[w])
                    xrw = sb.tile([P, hc], F32, tag=f"xrw{layer}")
                    nc.sync.dma_start(out=xrw[:], in_=shard_r[w * P:(w + 1) * P, :])
                    if layer == 1:
                        ta = sb.tile([P, M * (ED + 1)], F32, tag="arow")
                        nc.sync.dma_start(out=ta[:], in_=attr_rows[w])
                    Sw = swp.tile([P, M * P], F32, tag="Sw")
                    nc.sync.dma_start(out=Sw[:], in_=S_d[w])
                    Stw = swp.tile([P, M * P], F32, tag="Stw")
                    nc.sync.dma_start(out=Stw[:], in_=St_d[w])
                    for j in range(M):
                        t = w * M + j
                        S = Sw[:, j * P:(j + 1) * P]
                        St = Stw[:, j * P:(j + 1) * P]
                        sl = gat.tile([P, hc], F32, tag=f"sl{layer}")
                        nc.gpsimd.indirect_dma_start(
                            out=sl[:], out_offset=None, in_=full[:],
                            in_offset=bass.IndirectOffsetOnAxis(
                                ap=tsrc[:, t:t + 1], axis=0),
                        )
                        pm_ = psea.tile([P, hc], F32, tag="pea", space="PSUM")
                        nc.tensor.matmul(out=pm_[:], lhsT=St, rhs=xrw[:],
                                         start=True, stop=False)
                        nc.tensor.matmul(
                            out=pm_[:], lhsT=taT[:, j * P:(j + 1) * P],
                            rhs=tWe[:], start=False, stop=True,
                        )
                        m = gat.tile([P, hc], F32, tag=f"m{layer}")
                        nc.vector.tensor_add(out=m[:], in0=sl[:], in1=pm_[:])
                        msc = gat.tile([P, hc], F32, tag=f"msc{layer}")
                        nc.scalar.activation(out=msc[:], in_=m[:], func=AF.Copy,
                                             scale=NEG)
                        nc.vector.tensor_tensor(out=m[:], in0=m[:], in1=msc[:],
                                                op=OP.max)
                        tt = gat.tile([P, hc], F32, tag=f"tt{layer}")
                        nc.vector.tensor_mul(out=tt[:], in0=m[:], in1=tatt[:])
                        rhs = gat.tile([P, ew], F32, tag=f"rhs{layer}")
                        ex = rhs[:, hc:hc + heads]
                        nc.vector.tensor_reduce(
                            out=ex,
                            in_=tt[:].rearrange("p (h c) -> p h c", h=heads),
                            axis=AX.X, op=OP.add,
                        )
                        nc.scalar.activation(out=ex, in_=ex, func=AF.Exp)
                        nc.vector.tensor_tensor(
                            out=rhs[:, :hc].rearrange("p (h c) -> p h c", h=heads),
                            in0=sl[:].rearrange("p (h c) -> p h c", h=heads),
                            in1=ex.rearrange("p (h o) -> p h o", o=1)
                                .to_broadcast([P, heads, ch]),
                            op=OP.mult,
                        )
                        if layer == 1:
                            nc.scalar.copy(
                                out=rhs[:, hc + heads:hc + heads + ED + 1],
                                in_=ta[:, j * (ED + 1):(j + 1) * (ED + 1)])
                        nc.tensor.matmul(
                            out=pagg[:], lhsT=S, rhs=rhs[:],
                            start=(j == 0), stop=(j == M - 1),
                        )
                    # ---- window tail: mean-attr finish (layer 1 only)
                    if layer == 1:
                        cnt = sb.tile([P, 1], F32, tag="cnt")
                        nc.vector.tensor_scalar(
                            out=cnt[:], in0=pagg[:, hc + heads + ED:],
                            scalar1=1.0, scalar2=None, op0=OP.max,
                        )
                        rcc = sb.tile([P, 1], F32, tag="rcc")
                        nc.vector.reciprocal(out=rcc[:], in_=cnt[:])
                        lat = sb.tile([P, ED], F32, tag="lat")
                        nc.vector.tensor_scalar(
                            out=lat[:],
                            in0=pagg[:, hc + heads:hc + heads + ED],
                            scalar1=rcc[:, :1], scalar2=None, op0=OP.mult,
                        )
                        ptr = pg.tile([ED, P], F32, tag="pg", space="PSUM")
                        nc.tensor.transpose(out=ptr[:], in_=lat[:],
                                            identity=tident[:])
                        nc.scalar.copy(out=tlatT[:, w * P:(w + 1) * P], in_=ptr[:])
                    # ---- self-loop (dense) + normalize
                    xlw = sb.tile([P, hc], F32, tag=f"xlw{layer}")
                    nc.sync.dma_start(out=xlw[:], in_=loc_l[w * P:(w + 1) * P, :])
                    pel = psea.tile([P, hc], F32, tag="pea", space="PSUM")
                    nc.tensor.matmul(
                        out=pel[:], lhsT=tlatT[:, w * P:(w + 1) * P], rhs=tWe[:],
                        start=True, stop=True,
                    )
                    ml = sb.tile([P, hc], F32, tag=f"ml{layer}")
                    nc.vector.tensor_add(out=ml[:], in0=xlw[:], in1=xrw[:])
                    nc.vector.tensor_add(out=ml[:], in0=ml[:], in1=pel[:])
                    mlsc = sb.tile([P, hc], F32, tag=f"mlsc{layer}")
                    nc.scalar.activation(out=mlsc[:], in_=ml[:], func=AF.Copy,
                                         scale=NEG)
                    nc.vector.tensor_tensor(out=ml[:], in0=ml[:], in1=mlsc[:],
                                            op=OP.max)
                    nc.vector.tensor_mul(out=ml[:], in0=ml[:], in1=tatt[:])
                    exl = sb.tile([P, heads], F32, tag=f"exl{layer}")
                    nc.vector.tensor_reduce(
                        out=exl[:],
                        in_=ml[:].rearrange("p (h c) -> p h c", h=heads),
                        axis=AX.X, op=OP.add,
                    )
                    nc.scalar.activation(out=exl[:], in_=exl[:], func=AF.Exp)
                    den = sb.tile([P, heads], F32, tag=f"den{layer}")
                    nc.vector.tensor_add(out=den[:], in0=pagg[:, hc:hc + heads], in1=exl[:])
                    rec = sb.tile([P, heads], F32, tag=f"rec{layer}")
                    nc.vector.reciprocal(out=rec[:], in_=den[:])
                    hout = sb.tile([P, hc], F32, tag=f"h{layer}")
                    for hh in range(heads):
                        sli = slice(hh * ch, (hh + 1) * ch)
                        nc.scalar.activation(
                            out=hout[:, sli], in_=xlw[:, sli], func=AF.Copy,
                            scale=exl[:, hh:hh + 1],
                        )
                    nc.vector.tensor_add(out=hout[:], in0=hout[:], in1=pagg[:, :hc])
                    for hh in range(heads):
                        sli = slice(hh * ch, (hh + 1) * ch)
                        nc.scalar.activation(
                            out=hout[:, sli], in_=hout[:, sli], func=AF.Copy,
                            scale=rec[:, hh:hh + 1],
                        )
                    nc.vector.tensor_add(out=hout[:], in0=hout[:], in1=tgb[:])
                    if layer == 1:
                        tmin = sb.tile([P, hc], F32, tag="tmin")
                        nc.vector.tensor_scalar(
                            out=tmin[:], in0=hout[:], scalar1=0.0, scalar2=None,
                            op0=OP.min,
                        )
                        nc.scalar.activation(out=tmin[:], in_=tmin[:], func=AF.Exp)
                        helu = sb.tile([P, hc], F32, tag="helu")
                        nc.vector.tensor_scalar(
                            out=helu[:], in0=hout[:], scalar1=0.0, scalar2=-1.0,
                            op0=OP.max, op1=OP.add,
                        )
                        nc.vector.tensor_add(out=helu[:], in0=helu[:], in1=tmin[:])
                        ptr2 = pg.tile([HC, P], F32, tag="pg", space="PSUM")
                        nc.tensor.transpose(out=ptr2[:], in_=helu[:],
                                            identity=tident[:])
                        heluT = sb.tile([HC, P], F32, tag="heluT")
                        nc.scalar.copy(out=heluT[:], in_=ptr2[:])
                        p2 = pg.tile([P, 2 * OUT], F32, tag="pg", space="PSUM")
                        nc.tensor.matmul(out=p2[:], lhsT=heluT[:], rhs=tW2[:],
                                         start=True, stop=True)
                        so2 = sb.tile([P, 2 * OUT], F32, tag="so2")
                        nc.vector.tensor_add(out=so2[:], in0=p2[:], in1=tb2c[:])
                        nc.sync.dma_start(out=xl2_sh[w * P:(w + 1) * P, :],
                                          in_=so2[:, :OUT])
                        nc.sync.dma_start(out=xr2_sh[w * P:(w + 1) * P, :],
                                          in_=so2[:, OUT:])
                    else:
                        ptr3 = pg.tile([OUT, P], F32, tag="pg", space="PSUM")
                        nc.tensor.transpose(out=ptr3[:], in_=hout[:],
                                            identity=tident[:])
                        h2T = sb.tile([OUT, P], F32, tag="h2T")
                        nc.scalar.copy(out=h2T[:], in_=ptr3[:])
                        p3 = pg.tile([P, 2 * OUT], F32, tag="pg", space="PSUM")
                        nc.tensor.matmul(out=p3[:], lhsT=h2T[:], rhs=tWm[:],
                                         start=True, stop=True)
                        uo = sb.tile([P, OUT], F32, tag="uo")
                        nc.vector.tensor_add(out=uo[:], in0=p3[:, :OUT], in1=tbm1[:])
                        nc.sync.dma_start(out=u_sh[w * P:(w + 1) * P, :], in_=uo[:])
                        vo = sb.tile([P, OUT], F32, tag="vo")
                        nc.scalar.copy(out=vo[:], in_=p3[:, OUT:])
                        nc.sync.dma_start(out=v_sh[w * P:(w + 1) * P, :], in_=vo[:])

            gat_layer(1)
            allgather(xl2_sh, xl2_full)
            gat_layer(2)
            allgather(u_sh, u_full)

            # ================= Phase 4: edge MLP scores
            outsb = bulk.tile([P, NT], F32, tag="outsb")
            for w in range(nwin):
                vw = sb.tile([P, OUT], F32, tag="vw")
                nc.sync.dma_start(out=vw[:], in_=v_sh[w * P:(w + 1) * P, :])
                Stw = swp.tile([P, M * P], F32, tag="Stw")
                nc.sync.dma_start(out=Stw[:], in_=St_d[w])
                for j in range(M):
                    t = w * M + j
                    St = Stw[:, j * P:(j + 1) * P]
                    us = gat.tile([P, OUT], F32, tag="us")
                    nc.gpsimd.indirect_dma_start(
                        out=us[:], out_offset=None, in_=u_full[:],
                        in_offset=bass.IndirectOffsetOnAxis(
                            ap=tsrc[:, t:t + 1], axis=0),
                    )
                    pv = psea.tile([P, OUT], F32, tag="pea", space="PSUM")
                    nc.tensor.matmul(out=pv[:], lhsT=St, rhs=vw[:],
                                     start=True, stop=True)
                    z = gat.tile([P, OUT], F32, tag="z")
                    nc.vector.tensor_add(out=z[:], in0=us[:], in1=pv[:])
                    nc.scalar.activation(out=z[:], in_=z[:], func=AF.Relu)
                    nc.vector.tensor_mul(out=z[:], in0=z[:], in1=twm2[:])
                    val = gat.tile([P, 1], F32, tag="val")
                    nc.vector.tensor_reduce(out=val[:], in_=z[:], axis=AX.X,
                                            op=OP.add)
                    nc.vector.tensor_add(out=outsb[:, t:t + 1], in0=val[:],
                                         in1=tbm2[:])
            nc.sync.dma_start(out=outv[:], in_=outsb[:])
    return nc


def kernel(x, edge_index, edge_attr,
           Wl1, bl1, Wr1, br1, We1, att1, b1,
           Wl2, bl2, Wr2, br2, We2, att2, b2,
           Wm1, bm1, Wm2, bm2):
    x = np.asarray(x, np.float32)
    edge_index = np.asarray(edge_index, np.int32)
    edge_attr = np.asarray(edge_attr, np.float32)
    N = x.shape[0]
    E = edge_index.shape[1]

    cores, n_pad, npc, nwin, M = host_prep(x, edge_index, edge_attr, N)
    NT = nwin * M

    xp = np.zeros((n_pad, IN), np.float32)
    xp[:N] = x
    xT = np.ascontiguousarray(xp.T)                  # [IN, n_pad]

    def bc(v, width):
        v = np.asarray(v, np.float32).reshape(-1)
        return np.ascontiguousarray(np.broadcast_to(v[None, :width], (P, width)))

    W1cat = np.ascontiguousarray(np.concatenate([Wl1, Wr1], axis=1), np.float32)
    b1cat = bc(np.concatenate([np.asarray(bl1), np.asarray(br1)]), 2 * HC)
    att1f = bc(np.asarray(att1, np.float32).reshape(-1), HC)
    W2cat = np.ascontiguousarray(np.concatenate([Wl2, Wr2], axis=1), np.float32)
    b2cat = bc(np.concatenate([np.asarray(bl2), np.asarray(br2)]), 2 * OUT)
    att2f = bc(np.asarray(att2, np.float32).reshape(-1), OUT)
    Wm1 = np.asarray(Wm1, np.float32)
    WmCat = np.ascontiguousarray(
        np.concatenate([Wm1[:OUT, :], Wm1[OUT:, :]], axis=1))  # [OUT, 2*OUT]
    wm2b = bc(np.asarray(Wm2, np.float32).reshape(-1), OUT)
    ident = np.eye(P, dtype=np.float32)

    shared = dict(
        W1cat=W1cat, b1cat=b1cat, We1=np.asarray(We1, np.float32),
        att1=att1f, gb1=bc(b1, HC),
        W2cat=W2cat, b2cat=b2cat, We2=np.asarray(We2, np.float32),
        att2=att2f, gb2=bc(b2, OUT),
        WmCat=WmCat, bm1=bc(bm1, OUT), wm2=wm2b,
        bm2=bc(bm2, 1),
        ident=ident,
    )

    in_maps = []
    for cidx in range(NC_CORES):
        cd = cores[cidx]
        base = cidx * npc
        m = dict(shared)
        m["xT"] = np.ascontiguousarray(xT[:, base:base + npc])
        m["attr_rows"] = cd["attr_rows"]
        m["attrT"] = cd["attrT"]
        m["srcg"] = cd["srcg"]
        m["dstsh"] = cd["dstsh"]
        m["S_d"] = cd["S_d"]
        m["St_d"] = cd["St_d"]
        in_maps.append(m)

    nc = build_nc(nwin, M, n_pad, npc)
    res = run_bass_kernel_spmd(nc, in_maps, core_ids=list(range(NC_CORES)),
                               trace=_TRACE[0])
    _LAST[0] = res.exec_time_ns

    out = np.zeros((E, 1), np.float32)
    for cidx in range(NC_CORES):
        ov = res.results[cidx]["outv"]               # [P, NT]
        ov = ov.reshape(P, nwin, M).transpose(1, 2, 0)   # [nwin, M, P]
        om = cores[cidx]["outmap"]
        sel = om >= 0
        out[om[sel], 0] = ov[sel]
    return out



# revision 3
# speedup vs baseline: 1.0596x; 1.0596x over previous
"""GATv2 edge predictor on 8 TRN2 NeuronCores — v2.

Sharding: nodes degree-balanced into 392 windows of 128 (host permutation) so
every window holds <= M*128 edges; edges partitioned by dst window across the
8 cores.  Per-edge messages are computed in TRANSPOSED form ([channels, edges])
so the xr/ea/logit stages are single big bf16 matmuls per 512-edge group.
Layer-1 x[src] is delivered by the host in edge-slot order (no device gather);
layers 2 and the edge-MLP gather bf16 node rows via per-tile indirect DMA.
Segment softmax as in v1: exp without max-subtraction, one-hot scatter matmuls
into PSUM, dense self-loop tail per window.
"""
import math
import numpy as np

import concourse.bass as bass
import concourse.tile as tile
import concourse.mybir as mybir
from concourse.bass_utils import run_bass_kernel_spmd

# ---------------------------------------------------------------- wait patch
# This container's walrus build rejects >1 sync-wait command per instruction.
# Hoist extra waits onto single-wait NoOps on the same engine (engine streams
# execute in order, so gating is equivalent), and split the kernel-tail
# drain's per-proc waits the same way.
import bass_rust
from concourse.vector_clock import ScopedClock
from bass_rust import VectorClock as _RVC

_orig_commit = tile.TileContext._commit_and_lower
_ctr = [0]


def _split_commit(self, inst, original_block, old_bb_map, bb_to_exit_bb):
    si = getattr(inst, "sync_info", None)
    if si is not None:
        waits = list(si.on_wait)
        if len(waits) > 1:
            hoist = [w for w in waits if w.wait_reg is None]
            keep = [w for w in waits if w.wait_reg is not None]
            if not keep:
                keep = [hoist.pop()]
            for w in hoist:
                _ctr[0] += 1
                n = mybir.InstNoOp(name=f"TW-{_ctr[0]}", ins=[], outs=[])
                n.engine = inst.engine
                n.sync_info = bass_rust.SyncInfo(on_wait=[w], on_update=[])
                _orig_commit(self, n, original_block, old_bb_map, bb_to_exit_bb)
            inst.sync_info = bass_rust.SyncInfo(
                on_wait=keep, on_update=list(si.on_update)
            )
    return _orig_commit(self, inst, original_block, old_bb_map, bb_to_exit_bb)


def _patched_drain_and_barrier(self, tick_clock, wait_clock):
    ticks = list(tick_clock.global_clock)
    for i, t in enumerate(ticks):
        if t > 0:
            sub = [t if j == i else 0 for j in range(len(ticks))]
            nop_inst = self.nc.sync.nop(nofuse=True).ins
            wait_clock.add_sem_waits(nop_inst, ScopedClock({None: _RVC(sub)}))
    self.nc.sync.drain()
    self.nc.all_engine_barrier()
    assert self.sems is not None
    popped = self.nc._tile_sem_poison_stack.pop()
    assert popped is self._sem_poison
    self.nc.clear_and_free_semaphores(list(self.sems.allocated().values()))
    self.nc.all_engine_barrier()


tile.TileContext._commit_and_lower = _split_commit
tile.TileContext._drain_and_barrier = _patched_drain_and_barrier
# ------------------------------------------------------------ end wait patch

F32 = mybir.dt.float32
BF16 = mybir.dt.bfloat16
I32 = mybir.dt.int32
AF = mybir.ActivationFunctionType
OP = mybir.AluOpType
AX = mybir.AxisListType

NC_CORES = 8
P = 128
IN = 128
H = 2
C = 64
HC = H * C          # 128
OUT = 64
ED = 16
NEG = 0.2
G = 4               # tiles per group (512 edges)

_TRACE = [False]
_LAST = [None]


def _ceil_to(x, m):
    return ((x + m - 1) // m) * m


def _bf16(x):
    import jax.numpy as jnp
    return np.asarray(jnp.asarray(np.asarray(x, np.float32), jnp.bfloat16))


def host_prep(x, edge_index, edge_attr, n_nodes):
    """Structure prep: degree-balanced node permutation + per-core edge-slot
    arrays.  Returns (cores, n_pad, nodes_per_core, nwin, M)."""
    E = edge_index.shape[1]
    src = edge_index[0].astype(np.int64)
    dst = edge_index[1].astype(np.int64)
    x = np.asarray(x, np.float32)

    n_pad = _ceil_to(n_nodes, NC_CORES * P)          # 50176
    npc = n_pad // NC_CORES                          # 6272
    nwin = npc // P                                  # 49
    tot_win = NC_CORES * nwin                        # 392

    # ---- degree-balanced windowing: permute node ids so each window of 128
    # nodes has total in-degree <= CAP.
    deg = np.bincount(dst, minlength=n_pad).astype(np.int64)
    CAP = _ceil_to(max(1, int(math.ceil(E / tot_win))), P)   # 2048
    order_nodes = np.argsort(-deg, kind="stable")
    loads = np.zeros(tot_win, np.int64)
    counts = np.zeros(tot_win, np.int64)
    pid = np.zeros(n_pad, np.int64)
    import heapq
    heap = [(0, b) for b in range(tot_win)]
    heapq.heapify(heap)
    spill = []
    for n in order_nodes:
        d = deg[n]
        tmp = []
        placed = False
        while heap:
            load, b = heapq.heappop(heap)
            if counts[b] < P and (load + d <= CAP or d == 0):
                pid[n] = b * P + counts[b]
                counts[b] += 1
                loads[b] = load + d
                heapq.heappush(heap, (loads[b], b))
                placed = True
                break
            tmp.append((load, b))
        for it in tmp:
            heapq.heappush(heap, it)
        if not placed:
            spill.append(n)
    for n in spill:  # capacity exceeded somewhere: place least-loaded open bin
        cand = [b for b in range(tot_win) if counts[b] < P]
        b = min(cand, key=lambda bb: loads[bb])
        pid[n] = b * P + counts[b]
        counts[b] += 1
        loads[b] += deg[n]
    M = max(1, int(math.ceil(loads.max() / P)))

    psrc = pid[src]
    pdst = pid[dst]

    order = np.argsort(pdst, kind="stable")
    dsts = pdst[order]
    wbound = np.searchsorted(dsts, np.arange(tot_win + 1) * P)

    # inverse permutation for x: xperm[p] = x[orig node with pid p]
    inv = np.zeros(n_pad, np.int64)
    inv[pid] = np.arange(n_pad)
    xperm = np.zeros((n_pad, IN), np.float32)
    real = inv < n_nodes
    xperm[real] = x[inv[real]]

    cores = []
    for cidx in range(NC_CORES):
        srcg = np.zeros((P, nwin * M), np.int32)
        xe = np.zeros((nwin, M * P, IN), np.float32)
        attrT = np.zeros((nwin, ED, M * P), np.float32)
        attr_rows = np.zeros((nwin, P, M * (ED + 1)), np.float32)
        dstwin = np.full((nwin, M, P), -1.0, np.float32)
        outmap = np.full((nwin, M, P), -1, np.int64)
        for w in range(nwin):
            gw = cidx * nwin + w
            e0, e1 = wbound[gw], wbound[gw + 1]
            cnt = e1 - e0
            if cnt == 0:
                continue
            eids = order[e0:e1]
            j = np.arange(cnt) // P
            p = np.arange(cnt) % P
            srcg[p, w * M + j] = psrc[eids]
            xe[w, j * P + p, :] = x[src[eids]]
            a = edge_attr[eids]
            attrT[w][:, j * P + p] = a.T
            attr_rows[w, p[:, None],
                      (j * (ED + 1))[:, None] + np.arange(ED)[None, :]] = a
            attr_rows[w, p, j * (ED + 1) + ED] = 1.0
            dstwin[w, j, p] = (pdst[eids] - gw * P).astype(np.float32)
            outmap[w, j, p] = eids
        oneh = (dstwin[..., None] == np.arange(P, dtype=np.float32)
                ).astype(np.float32)           # [nwin, M, Pe, Pn]
        S_d = np.ascontiguousarray(
            oneh.transpose(0, 2, 1, 3).reshape(nwin, P, M * P))
        St_d = np.ascontiguousarray(
            oneh.transpose(0, 3, 1, 2).reshape(nwin, P, M * P))
        xeT = np.ascontiguousarray(xe.transpose(0, 2, 1))   # [nwin, IN, M*P]
        cores.append(dict(
            S_d=_bf16(S_d), St_d=_bf16(St_d),
            xeT=_bf16(xeT), attrT=_bf16(attrT), attr_rows=_bf16(attr_rows),
            srcg=srcg, outmap=outmap,
            xpermT=np.ascontiguousarray(
                xperm[cidx * npc:(cidx + 1) * npc].T),       # [IN, npc] f32
        ))
    return cores, n_pad, npc, nwin, M


def build_nc(nwin, M, n_pad, npc, nocoll=False):
    nc = bass.Bass()
    NT = nwin * M
    NG = (M + G - 1) // G
    EW1 = HC + H + ED + 1          # pagg layer1 cols
    EW2 = OUT + 1

    def param(name, shape, dt=F32):
        return nc.declare_dram_parameter(name, list(shape), dt, isOutput=False)

    xT = param("xT", [IN, npc], BF16)
    xeT = param("xeT", [nwin, IN, M * P], BF16)
    S_d = param("S_d", [nwin, P, M * P], BF16)
    St_d = param("St_d", [nwin, P, M * P], BF16)
    attrT = param("attrT", [nwin, ED, M * P], BF16)
    attr_rows = param("attr_rows", [nwin, P, M * (ED + 1)], BF16)
    srcg = param("srcg", [P, NT], I32)
    W1cat = param("W1cat", [IN, 2 * HC], BF16)
    b1cat = param("b1cat", [P, 2 * HC])
    We1 = param("We1", [ED, HC], BF16)
    attS1 = param("attS1", [HC, H], BF16)
    att1bc = param("att1bc", [P, HC])
    gb1 = param("gb1", [P, HC])
    W2cat = param("W2cat", [HC, 2 * OUT], BF16)
    b2cat = param("b2cat", [P, 2 * OUT])
    We2 = param("We2", [ED, OUT], BF16)
    att2col = param("att2col", [OUT, 1], BF16)
    att2bc = param("att2bc", [P, OUT])
    gb2 = param("gb2", [P, OUT])
    WmCat = param("WmCat", [OUT, 2 * OUT], BF16)
    bmv = param("bmv", [P, OUT])
    wm2col = param("wm2col", [OUT, 1], BF16)
    bm2 = param("bm2", [P, 1])
    identb = param("identb", [P, P], BF16)
    identf = param("identf", [P, P])
    outv = nc.declare_dram_parameter("outv", [nwin, M * P], F32, isOutput=True)

    with tile.TileContext(nc) as tc:
        with (
            tc.tile_pool(name="const", bufs=1) as cp,
            tc.tile_pool(name="bulk", bufs=1) as bulk,
            tc.tile_pool(name="win", bufs=3) as wp,
            tc.tile_pool(name="grp", bufs=3) as gp,
            tc.tile_pool(name="tl", bufs=4) as tp,
            tc.tile_pool(name="gat", bufs=8) as gat,
            tc.tile_pool(name="psA", bufs=2, space="PSUM") as psA,
            tc.tile_pool(name="psB", bufs=2, space="PSUM") as psB,
            tc.tile_pool(name="psC", bufs=3, space="PSUM") as psC,
            tc.tile_pool(name="psD", bufs=1, space="PSUM") as psD,
            tc.tile_pool(name="dram", bufs=1, space="DRAM") as dram,
        ):
            # ---- constants
            def ld(ap, shape, dt=F32):
                t = cp.tile(list(shape), dt, tag=f"c{ld.i}")
                ld.i += 1
                nc.sync.dma_start(out=t[:], in_=ap[:])
                return t
            ld.i = 0

            tW1 = ld(W1cat, [IN, 2 * HC], BF16)
            tb1c = ld(b1cat, [P, 2 * HC])
            tWe1 = ld(We1, [ED, HC], BF16)
            tattS1 = ld(attS1, [HC, H], BF16)
            tatt1bc = ld(att1bc, [P, HC])
            tgb1 = ld(gb1, [P, HC])
            tW2 = ld(W2cat, [HC, 2 * OUT], BF16)
            tb2c = ld(b2cat, [P, 2 * OUT])
            tWe2 = ld(We2, [ED, OUT], BF16)
            tatt2c = ld(att2col, [OUT, 1], BF16)
            tatt2bc = ld(att2bc, [P, OUT])
            tgb2 = ld(gb2, [P, OUT])
            tWm = ld(WmCat, [OUT, 2 * OUT], BF16)
            tbmv = ld(bmv, [P, OUT])
            twm2 = ld(wm2col, [OUT, 1], BF16)
            tbm2 = ld(bm2, [P, 1])
            tidb = ld(identb, [P, P], BF16)
            tidf = ld(identf, [P, P])

            txT = bulk.tile([IN, npc], BF16, tag="xT")
            nc.sync.dma_start(out=txT[:], in_=xT[:])
            tsrc = bulk.tile([P, NT], I32, tag="srcg")
            nc.sync.dma_start(out=tsrc[:], in_=srcg[:])

            # node-shard SBUF tables (windows side by side)
            xl_sb = bulk.tile([P, nwin * HC], BF16, tag="xl_sb")
            xr_sb = bulk.tile([P, nwin * HC], BF16, tag="xr_sb")
            xl2_sb = bulk.tile([P, nwin * OUT], BF16, tag="xl2_sb")
            xr2_sb = bulk.tile([P, nwin * OUT], BF16, tag="xr2_sb")
            v_sb = bulk.tile([P, nwin * OUT], BF16, tag="v_sb")
            latT_sb = bulk.tile([ED, nwin * P], BF16, tag="latT_sb")

            # internal DRAM
            xl2_sh = dram.tile([npc, OUT], BF16, tag="xl2_sh")
            xl2_full = dram.tile([n_pad, OUT], BF16, tag="xl2_full")
            u_sh = dram.tile([npc, OUT], BF16, tag="u_sh")
            u_full = dram.tile([n_pad, OUT], BF16, tag="u_full")

            def allgather(shard, full):
                if nocoll:
                    nc.sync.dma_start(out=full[0:npc, :], in_=shard[:])
                else:
                    nc.gpsimd.collective_compute(
                        "AllGather", OP.bypass,
                        replica_groups=[list(range(NC_CORES))],
                        ins=[shard.opt()], outs=[full.opt()],
                    )

            # ================= Phase 0: xl/xr projections into SBUF
            for w in range(nwin):
                pm = psA.tile([P, 2 * HC], F32, tag="psA", space="PSUM")
                nc.tensor.matmul(out=pm[:], lhsT=txT[:, w * P:(w + 1) * P],
                                 rhs=tW1[:], start=True, stop=True)
                nc.scalar.copy(out=xl_sb[:, w * HC:(w + 1) * HC],
                               in_=pm[:, :HC])
                nc.vector.tensor_add(out=xr_sb[:, w * HC:(w + 1) * HC],
                                     in0=pm[:, HC:], in1=tb1c[:, HC:])

            # ================= Layer 1 (no gathers)
            for w in range(nwin):
                txe = wp.tile([IN, M * P], BF16, tag="xe")
                nc.sync.dma_start(out=txe[:], in_=xeT[w])
                tS = wp.tile([P, M * P], BF16, tag="S")
                nc.sync.dma_start(out=tS[:], in_=S_d[w])
                tSt = wp.tile([P, M * P], BF16, tag="St")
                nc.sync.dma_start(out=tSt[:], in_=St_d[w])
                taT = wp.tile([ED, M * P], BF16, tag="aT")
                nc.sync.dma_start(out=taT[:], in_=attrT[w])
                tar = wp.tile([P, M * (ED + 1)], BF16, tag="ar")
                nc.sync.dma_start(out=tar[:], in_=attr_rows[w])
                xlw = xl_sb[:, w * HC:(w + 1) * HC]
                xrw = xr_sb[:, w * HC:(w + 1) * HC]
                pagg = psB.tile([P, EW1], F32, tag="psB", space="PSUM")
                for g in range(NG):
                    j0 = g * G
                    jn = min(G, M - j0)
                    W = jn * P
                    es = slice(j0 * P, j0 * P + W)
                    mT = psA.tile([HC, G * P], F32, tag="psA", space="PSUM")
                    nc.tensor.matmul(out=mT[:, :W], lhsT=tW1[:, :HC],
                                     rhs=txe[:, es], start=True, stop=False)
                    nc.tensor.matmul(out=mT[:, :W], lhsT=xrw,
                                     rhs=tSt[:, es], start=False, stop=False)
                    nc.tensor.matmul(out=mT[:, :W], lhsT=tWe1[:],
                                     rhs=taT[:, es], start=False, stop=True)
                    mlk = gp.tile([HC, G * P], BF16, tag="mlk")
                    nc.scalar.activation(out=mlk[:, :W], in_=mT[:, :W],
                                         func=AF.Prelu, alpha=NEG)
                    lg = psD.tile([H, G * P], F32, tag="psD", space="PSUM")
                    nc.tensor.matmul(out=lg[:, :W], lhsT=tattS1[:],
                                     rhs=mlk[:, :W], start=True, stop=True)
                    exT = gp.tile([H, G * P], BF16, tag="exT")
                    nc.scalar.activation(out=exT[:, :W], in_=lg[:, :W],
                                         func=AF.Exp)
                    exg = psC.tile([P, HC], F32, tag="psC", space="PSUM")
                    for j in range(j0, j0 + jn):
                        jj = j - j0
                        nc.tensor.matmul(
                            out=exg[:, jj * H:(jj + 1) * H],
                            lhsT=exT[:, jj * P:(jj + 1) * P],
                            rhs=tidb[0:H, 0:H], start=True, stop=True)
                    exsb = gat.tile([P, G * H], BF16, tag="exsb")
                    nc.scalar.copy(out=exsb[:, :jn * H], in_=exg[:, :jn * H])
                    for j in range(j0, j0 + jn):
                        jj = j - j0
                        tsl = slice(j * P, (j + 1) * P)
                        slp = psC.tile([P, HC], F32, tag="psC", space="PSUM")
                        nc.tensor.matmul(out=slp[:], lhsT=txe[:, tsl],
                                         rhs=tW1[:, :HC], start=True, stop=True)
                        rv = gat.tile([P, HC], BF16, tag="rv")
                        nc.vector.tensor_tensor(
                            out=rv[:].rearrange("p (h c) -> p h c", h=H),
                            in0=slp[:].rearrange("p (h c) -> p h c", h=H),
                            in1=exsb[:, jj * H:(jj + 1) * H]
                                .rearrange("p (h o) -> p h o", o=1)
                                .to_broadcast([P, H, C]),
                            op=OP.mult)
                        nc.tensor.matmul(out=pagg[:, :HC], lhsT=tS[:, tsl],
                                         rhs=rv[:],
                                         start=(j == 0), stop=(j == M - 1))
                        nc.tensor.matmul(out=pagg[:, HC:HC + H],
                                         lhsT=tS[:, tsl],
                                         rhs=exsb[:, jj * H:(jj + 1) * H],
                                         start=(j == 0), stop=(j == M - 1))
                        nc.tensor.matmul(
                            out=pagg[:, HC + H:], lhsT=tS[:, tsl],
                            rhs=tar[:, j * (ED + 1):(j + 1) * (ED + 1)],
                            start=(j == 0), stop=(j == M - 1))
                # ---- window tail
                cnt = tp.tile([P, 1], F32, tag="cnt")
                nc.vector.tensor_scalar(out=cnt[:], in0=pagg[:, EW1 - 1:],
                                        scalar1=1.0, scalar2=None, op0=OP.max)
                rcc = tp.tile([P, 1], F32, tag="rcc")
                nc.vector.reciprocal(out=rcc[:], in_=cnt[:])
                lat = tp.tile([P, ED], F32, tag="lat")
                nc.vector.tensor_scalar(out=lat[:],
                                        in0=pagg[:, HC + H:HC + H + ED],
                                        scalar1=rcc[:, :1], scalar2=None,
                                        op0=OP.mult)
                ptr = psD.tile([ED, P], F32, tag="psD", space="PSUM")
                nc.tensor.matmul(out=ptr[:], lhsT=lat[:], rhs=tidf[:],
                                 start=True, stop=True)
                nc.scalar.copy(out=latT_sb[:, w * P:(w + 1) * P], in_=ptr[:])
                pel = psC.tile([P, HC], F32, tag="psC", space="PSUM")
                nc.tensor.matmul(out=pel[:],
                                 lhsT=latT_sb[:, w * P:(w + 1) * P],
                                 rhs=tWe1[:], start=True, stop=True)
                ml = tp.tile([P, HC], F32, tag="ml")
                nc.vector.tensor_add(out=ml[:], in0=xlw, in1=xrw)
                nc.vector.tensor_add(out=ml[:], in0=ml[:], in1=pel[:])
                mlk2 = tp.tile([P, HC], F32, tag="mlk2")
                nc.scalar.activation(out=mlk2[:], in_=ml[:], func=AF.Prelu,
                                     alpha=NEG)
                nc.vector.tensor_mul(out=mlk2[:], in0=mlk2[:], in1=tatt1bc[:])
                exl = tp.tile([P, H], F32, tag="exl")
                nc.vector.tensor_reduce(
                    out=exl[:], in_=mlk2[:].rearrange("p (h c) -> p h c", h=H),
                    axis=AX.X, op=OP.add)
                nc.scalar.activation(out=exl[:], in_=exl[:], func=AF.Exp)
                den = tp.tile([P, H], F32, tag="den")
                nc.vector.tensor_add(out=den[:], in0=pagg[:, HC:HC + H],
                                     in1=exl[:])
                rec = tp.tile([P, H], F32, tag="rec")
                nc.vector.reciprocal(out=rec[:], in_=den[:])
                hout = tp.tile([P, HC], F32, tag="hout")
                for hh in range(H):
                    sli = slice(hh * C, (hh + 1) * C)
                    nc.scalar.activation(out=hout[:, sli], in_=xlw[:, sli],
                                         func=AF.Copy,
                                         scale=exl[:, hh:hh + 1])
                nc.vector.tensor_add(out=hout[:], in0=hout[:],
                                     in1=pagg[:, :HC])
                for hh in range(H):
                    sli = slice(hh * C, (hh + 1) * C)
                    nc.scalar.activation(out=hout[:, sli], in_=hout[:, sli],
                                         func=AF.Copy,
                                         scale=rec[:, hh:hh + 1])
                nc.vector.tensor_add(out=hout[:], in0=hout[:], in1=tgb1[:])
                # ELU
                tmin = tp.tile([P, HC], F32, tag="tmin")
                nc.vector.tensor_scalar(out=tmin[:], in0=hout[:], scalar1=0.0,
                                        scalar2=None, op0=OP.min)
                nc.scalar.activation(out=tmin[:], in_=tmin[:], func=AF.Exp)
                helu = tp.tile([P, HC], BF16, tag="helu")
                nc.vector.tensor_scalar(out=helu[:], in0=hout[:], scalar1=0.0,
                                        scalar2=-1.0, op0=OP.max, op1=OP.add)
                nc.vector.tensor_add(out=helu[:], in0=helu[:], in1=tmin[:])
                ptr2 = psD.tile([HC, P], F32, tag="psD", space="PSUM")
                nc.tensor.matmul(out=ptr2[:], lhsT=helu[:], rhs=tidb[:],
                                 start=True, stop=True)
                heluT = tp.tile([HC, P], BF16, tag="heluT")
                nc.scalar.copy(out=heluT[:], in_=ptr2[:])
                p2 = psD.tile([P, 2 * OUT], F32, tag="psD", space="PSUM")
                nc.tensor.matmul(out=p2[:], lhsT=heluT[:], rhs=tW2[:],
                                 start=True, stop=True)
                nc.scalar.copy(out=xl2_sb[:, w * OUT:(w + 1) * OUT],
                               in_=p2[:, :OUT])
                nc.sync.dma_start(out=xl2_sh[w * P:(w + 1) * P, :],
                                  in_=xl2_sb[:, w * OUT:(w + 1) * OUT])
                nc.vector.tensor_add(out=xr2_sb[:, w * OUT:(w + 1) * OUT],
                                     in0=p2[:, OUT:], in1=tb2c[:, OUT:])

            allgather(xl2_sh, xl2_full)

            # ================= Layer 2 (gathers xl2 rows)
            for w in range(nwin):
                tS = wp.tile([P, M * P], BF16, tag="S")
                nc.sync.dma_start(out=tS[:], in_=S_d[w])
                tSt = wp.tile([P, M * P], BF16, tag="St")
                nc.sync.dma_start(out=tSt[:], in_=St_d[w])
                taT = wp.tile([ED, M * P], BF16, tag="aT")
                nc.sync.dma_start(out=taT[:], in_=attrT[w])
                xl2w = xl2_sb[:, w * OUT:(w + 1) * OUT]
                xr2w = xr2_sb[:, w * OUT:(w + 1) * OUT]
                pagg = psB.tile([P, EW2], F32, tag="psB", space="PSUM")
                for g in range(NG):
                    j0 = g * G
                    jn = min(G, M - j0)
                    W = jn * P
                    es = slice(j0 * P, j0 * P + W)
                    sls = []
                    mT = psA.tile([OUT, G * P], F32, tag="psA", space="PSUM")
                    for j in range(j0, j0 + jn):
                        jj = j - j0
                        t = w * M + j
                        sl = gat.tile([P, OUT], BF16, tag="sl2")
                        nc.gpsimd.indirect_dma_start(
                            out=sl[:], out_offset=None, in_=xl2_full[:],
                            in_offset=bass.IndirectOffsetOnAxis(
                                ap=tsrc[:, t:t + 1], axis=0))
                        sls.append(sl)
                        nc.tensor.matmul(out=mT[:, jj * P:(jj + 1) * P],
                                         lhsT=sl[:], rhs=tidb[:],
                                         start=True, stop=False)
                    nc.tensor.matmul(out=mT[:, :W], lhsT=xr2w,
                                     rhs=tSt[:, es], start=False, stop=False)
                    nc.tensor.matmul(out=mT[:, :W], lhsT=tWe2[:],
                                     rhs=taT[:, es], start=False, stop=True)
                    mlk = gp.tile([OUT, G * P], BF16, tag="mlk")
                    nc.scalar.activation(out=mlk[:, :W], in_=mT[:, :W],
                                         func=AF.Prelu, alpha=NEG)
                    lg = psD.tile([1, G * P], F32, tag="psD", space="PSUM")
                    nc.tensor.matmul(out=lg[:, :W], lhsT=tatt2c[:],
                                     rhs=mlk[:, :W], start=True, stop=True)
                    exT = gp.tile([1, G * P], BF16, tag="exT")
                    nc.scalar.activation(out=exT[:, :W], in_=lg[:, :W],
                                         func=AF.Exp)
                    exg = psC.tile([P, HC], F32, tag="psC", space="PSUM")
                    for j in range(j0, j0 + jn):
                        jj = j - j0
                        nc.tensor.matmul(
                            out=exg[:, jj:jj + 1],
                            lhsT=exT[:, jj * P:(jj + 1) * P],
                            rhs=tidb[0:1, 0:1], start=True, stop=True)
                    exsb = gat.tile([P, G], BF16, tag="exsb")
                    nc.scalar.copy(out=exsb[:, :jn], in_=exg[:, :jn])
                    for j in range(j0, j0 + jn):
                        jj = j - j0
                        tsl = slice(j * P, (j + 1) * P)
                        rv = gat.tile([P, OUT], BF16, tag="rv")
                        nc.vector.tensor_tensor(
                            out=rv[:], in0=sls[jj][:],
                            in1=exsb[:, jj:jj + 1].to_broadcast([P, OUT]),
                            op=OP.mult)
                        nc.tensor.matmul(out=pagg[:, :OUT], lhsT=tS[:, tsl],
                                         rhs=rv[:],
                                         start=(j == 0), stop=(j == M - 1))
                        nc.tensor.matmul(out=pagg[:, OUT:], lhsT=tS[:, tsl],
                                         rhs=exsb[:, jj:jj + 1],
                                         start=(j == 0), stop=(j == M - 1))
                # ---- tail
                pel = psC.tile([P, OUT], F32, tag="psC", space="PSUM")
                nc.tensor.matmul(out=pel[:],
                                 lhsT=latT_sb[:, w * P:(w + 1) * P],
                                 rhs=tWe2[:], start=True, stop=True)
                ml = tp.tile([P, OUT], F32, tag="ml2")
                nc.vector.tensor_add(out=ml[:], in0=xl2w, in1=xr2w)
                nc.vector.tensor_add(out=ml[:], in0=ml[:], in1=pel[:])
                mlk2 = tp.tile([P, OUT], F32, tag="mlk22")
                nc.scalar.activation(out=mlk2[:], in_=ml[:], func=AF.Prelu,
                                     alpha=NEG)
                nc.vector.tensor_mul(out=mlk2[:], in0=mlk2[:], in1=tatt2bc[:])
                exl = tp.tile([P, 1], F32, tag="exl2")
                nc.vector.tensor_reduce(out=exl[:], in_=mlk2[:], axis=AX.X,
                                        op=OP.add)
                nc.scalar.activation(out=exl[:], in_=exl[:], func=AF.Exp)
                den = tp.tile([P, 1], F32, tag="den2")
                nc.vector.tensor_add(out=den[:], in0=pagg[:, OUT:], in1=exl[:])
                rec = tp.tile([P, 1], F32, tag="rec2")
                nc.vector.reciprocal(out=rec[:], in_=den[:])
                hout = tp.tile([P, OUT], F32, tag="hout2")
                nc.scalar.activation(out=hout[:], in_=xl2w, func=AF.Copy,
                                     scale=exl[:, :1])
                nc.vector.tensor_add(out=hout[:], in0=hout[:],
                                     in1=pagg[:, :OUT])
                nc.scalar.activation(out=hout[:], in_=hout[:], func=AF.Copy,
                                     scale=rec[:, :1])
                houtb = tp.tile([P, OUT], BF16, tag="houtb")
                nc.vector.tensor_add(out=houtb[:], in0=hout[:], in1=tgb2[:])
                ptr3 = psD.tile([OUT, P], F32, tag="psD", space="PSUM")
                nc.tensor.matmul(out=ptr3[:], lhsT=houtb[:], rhs=tidb[:],
                                 start=True, stop=True)
                h2T = tp.tile([OUT, P], BF16, tag="h2T")
                nc.scalar.copy(out=h2T[:], in_=ptr3[:])
                p3 = psD.tile([P, 2 * OUT], F32, tag="psD", space="PSUM")
                nc.tensor.matmul(out=p3[:], lhsT=h2T[:], rhs=tWm[:],
                                 start=True, stop=True)
                uo = tp.tile([P, OUT], BF16, tag="uo")
                nc.scalar.copy(out=uo[:], in_=p3[:, :OUT])
                nc.sync.dma_start(out=u_sh[w * P:(w + 1) * P, :], in_=uo[:])
                nc.vector.tensor_add(out=v_sb[:, w * OUT:(w + 1) * OUT],
                                     in0=p3[:, OUT:], in1=tbmv[:])

            allgather(u_sh, u_full)

            # ================= Phase 4: edge MLP scores
            for w in range(nwin):
                tSt = wp.tile([P, M * P], BF16, tag="St")
                nc.sync.dma_start(out=tSt[:], in_=St_d[w])
                vw = v_sb[:, w * OUT:(w + 1) * OUT]
                outsb = gp.tile([1, M * P], F32, tag="outsb")
                for g in range(NG):
                    j0 = g * G
                    jn = min(G, M - j0)
                    W = jn * P
                    es = slice(j0 * P, j0 * P + W)
                    qT = psA.tile([OUT, G * P], F32, tag="psA", space="PSUM")
                    for j in range(j0, j0 + jn):
                        jj = j - j0
                        t = w * M + j
                        us = gat.tile([P, OUT], BF16, tag="us")
                        nc.gpsimd.indirect_dma_start(
                            out=us[:], out_offset=None, in_=u_full[:],
                            in_offset=bass.IndirectOffsetOnAxis(
                                ap=tsrc[:, t:t + 1], axis=0))
                        nc.tensor.matmul(out=qT[:, jj * P:(jj + 1) * P],
                                         lhsT=us[:], rhs=tidb[:],
                                         start=True, stop=False)
                    nc.tensor.matmul(out=qT[:, :W], lhsT=vw,
                                     rhs=tSt[:, es], start=False, stop=True)
                    zT = gp.tile([OUT, G * P], BF16, tag="zT")
                    nc.scalar.activation(out=zT[:, :W], in_=qT[:, :W],
                                         func=AF.Relu)
                    sc = psD.tile([1, G * P], F32, tag="psD", space="PSUM")
                    nc.tensor.matmul(out=sc[:, :W], lhsT=twm2[:],
                                     rhs=zT[:, :W], start=True, stop=True)
                    nc.vector.tensor_scalar(
                        out=outsb[:, j0 * P:j0 * P + W], in0=sc[:, :W],
                        scalar1=tbm2[:1, :1], scalar2=None, op0=OP.add)
                nc.sync.dma_start(out=outv[w], in_=outsb[:])
    return nc


def kernel(x, edge_index, edge_attr,
           Wl1, bl1, Wr1, br1, We1, att1, b1,
           Wl2, bl2, Wr2, br2, We2, att2, b2,
           Wm1, bm1, Wm2, bm2):
    x = np.asarray(x, np.float32)
    edge_index = np.asarray(edge_index, np.int32)
    edge_attr = np.asarray(edge_attr, np.float32)
    N = x.shape[0]
    E = edge_index.shape[1]

    cores, n_pad, npc, nwin, M = host_prep(x, edge_index, edge_attr, N)
    f32 = lambda a: np.asarray(a, np.float32)

    def bc(v, width):
        v = np.asarray(v, np.float32).reshape(-1)
        return np.ascontiguousarray(np.broadcast_to(v[None, :width], (P, width)))

    W1cat = _bf16(np.concatenate([f32(Wl1), f32(Wr1)], axis=1))
    b1cat = bc(np.concatenate([np.zeros(HC, np.float32),
                               f32(bl1) + f32(br1)]), 2 * HC)
    att1f = f32(att1).reshape(H, C)
    attS1 = np.zeros((HC, H), np.float32)
    for hh in range(H):
        attS1[hh * C:(hh + 1) * C, hh] = att1f[hh]
    W2cat = _bf16(np.concatenate([f32(Wl2), f32(Wr2)], axis=1))
    b2cat = bc(np.concatenate([np.zeros(OUT, np.float32),
                               f32(bl2) + f32(br2)]), 2 * OUT)
    Wm1f = f32(Wm1)
    WmCat = _bf16(np.concatenate([Wm1f[:OUT, :], Wm1f[OUT:, :]], axis=1))

    shared = dict(
        W1cat=W1cat, b1cat=b1cat, We1=_bf16(We1), attS1=_bf16(attS1),
        att1bc=bc(att1f.reshape(-1), HC), gb1=bc(f32(b1) + f32(bl1), HC),
        W2cat=W2cat, b2cat=b2cat, We2=_bf16(We2),
        att2col=_bf16(f32(att2).reshape(OUT, 1)),
        att2bc=bc(f32(att2).reshape(-1), OUT),
        gb2=bc(f32(b2) + f32(bl2), OUT),
        WmCat=WmCat, bmv=bc(bm1, OUT),
        wm2col=_bf16(f32(Wm2).reshape(OUT, 1)),
        bm2=bc(bm2, 1),
        identb=_bf16(np.eye(P, dtype=np.float32)),
        identf=np.eye(P, dtype=np.float32),
    )

    in_maps = []
    for cidx in range(NC_CORES):
        cd = cores[cidx]
        m = dict(shared)
        m["xT"] = _bf16(cd["xpermT"])
        m["xeT"] = cd["xeT"]
        m["S_d"] = cd["S_d"]
        m["St_d"] = cd["St_d"]
        m["attrT"] = cd["attrT"]
        m["attr_rows"] = cd["attr_rows"]
        m["srcg"] = cd["srcg"]
        in_maps.append(m)

    nc = build_nc(nwin, M, n_pad, npc)
    res = run_bass_kernel_spmd(nc, in_maps, core_ids=list(range(NC_CORES)),
                               trace=_TRACE[0])
    _LAST[0] = res.exec_time_ns

    out = np.zeros((E, 1), np.float32)
    for cidx in range(NC_CORES):
        ov = np.asarray(res.results[cidx]["outv"], np.float32)  # [nwin, M*P]
        ov = ov.reshape(nwin, M, P)
        om = cores[cidx]["outmap"]
        sel = om >= 0
        out[om[sel], 0] = ov[sel]
    return out


# revision 5
# speedup vs baseline: 1.0982x; 1.0364x over previous
"""GATv2 edge predictor on 8 TRN2 NeuronCores — v2.

Sharding: nodes degree-balanced into 392 windows of 128 (host permutation) so
every window holds <= M*128 edges; edges partitioned by dst window across the
8 cores.  Per-edge messages are computed in TRANSPOSED form ([channels, edges])
so the xr/ea/logit stages are single big bf16 matmuls per 512-edge group.
Layer-1 x[src] is delivered by the host in edge-slot order (no device gather);
layers 2 and the edge-MLP gather bf16 node rows via per-tile indirect DMA.
Segment softmax as in v1: exp without max-subtraction, one-hot scatter matmuls
into PSUM, dense self-loop tail per window.
"""
import math
import numpy as np

import concourse.bass as bass
import concourse.tile as tile
import concourse.mybir as mybir
from concourse.bass_utils import run_bass_kernel_spmd

# ---------------------------------------------------------------- wait patch
# This container's walrus build rejects >1 sync-wait command per instruction.
# Hoist extra waits onto single-wait NoOps on the same engine (engine streams
# execute in order, so gating is equivalent), and split the kernel-tail
# drain's per-proc waits the same way.
import bass_rust
from concourse.vector_clock import ScopedClock
from bass_rust import VectorClock as _RVC

_orig_commit = tile.TileContext._commit_and_lower
_ctr = [0]


def _split_commit(self, inst, original_block, old_bb_map, bb_to_exit_bb):
    si = getattr(inst, "sync_info", None)
    if si is not None:
        waits = list(si.on_wait)
        if len(waits) > 1:
            hoist = [w for w in waits if w.wait_reg is None]
            keep = [w for w in waits if w.wait_reg is not None]
            if not keep:
                keep = [hoist.pop()]
            for w in hoist:
                _ctr[0] += 1
                n = mybir.InstNoOp(name=f"TW-{_ctr[0]}", ins=[], outs=[])
                n.engine = inst.engine
                n.sync_info = bass_rust.SyncInfo(on_wait=[w], on_update=[])
                _orig_commit(self, n, original_block, old_bb_map, bb_to_exit_bb)
            inst.sync_info = bass_rust.SyncInfo(
                on_wait=keep, on_update=list(si.on_update)
            )
    return _orig_commit(self, inst, original_block, old_bb_map, bb_to_exit_bb)


def _patched_drain_and_barrier(self, tick_clock, wait_clock):
    ticks = list(tick_clock.global_clock)
    for i, t in enumerate(ticks):
        if t > 0:
            sub = [t if j == i else 0 for j in range(len(ticks))]
            nop_inst = self.nc.sync.nop(nofuse=True).ins
            wait_clock.add_sem_waits(nop_inst, ScopedClock({None: _RVC(sub)}))
    self.nc.sync.drain()
    self.nc.all_engine_barrier()
    assert self.sems is not None
    popped = self.nc._tile_sem_poison_stack.pop()
    assert popped is self._sem_poison
    self.nc.clear_and_free_semaphores(list(self.sems.allocated().values()))
    self.nc.all_engine_barrier()


tile.TileContext._commit_and_lower = _split_commit
tile.TileContext._drain_and_barrier = _patched_drain_and_barrier
# ------------------------------------------------------------ end wait patch

F32 = mybir.dt.float32
BF16 = mybir.dt.bfloat16
I32 = mybir.dt.int32
AF = mybir.ActivationFunctionType
OP = mybir.AluOpType
AX = mybir.AxisListType

NC_CORES = 8
P = 128
IN = 128
H = 2
C = 64
HC = H * C          # 128
OUT = 64
ED = 16
NEG = 0.2
G = 4               # tiles per group (512 edges)

_TRACE = [False]
_LAST = [None]


def _ceil_to(x, m):
    return ((x + m - 1) // m) * m


def _bf16(x):
    import jax.numpy as jnp
    return np.asarray(jnp.asarray(np.asarray(x, np.float32), jnp.bfloat16))


def host_prep(x, edge_index, edge_attr, n_nodes):
    """Structure prep: degree-balanced node permutation + per-core edge-slot
    arrays.  Returns (cores, n_pad, nodes_per_core, nwin, M)."""
    E = edge_index.shape[1]
    src = edge_index[0].astype(np.int64)
    dst = edge_index[1].astype(np.int64)
    x = np.asarray(x, np.float32)

    n_pad = _ceil_to(n_nodes, NC_CORES * P)          # 50176
    npc = n_pad // NC_CORES                          # 6272
    nwin = npc // P                                  # 49
    tot_win = NC_CORES * nwin                        # 392

    # ---- degree-balanced windowing: permute node ids so each window of 128
    # nodes has total in-degree <= CAP.
    deg = np.bincount(dst, minlength=n_pad).astype(np.int64)
    CAP = _ceil_to(max(1, int(math.ceil(E / tot_win))), P)   # 2048
    order_nodes = np.argsort(-deg, kind="stable")
    loads = np.zeros(tot_win, np.int64)
    counts = np.zeros(tot_win, np.int64)
    pid = np.zeros(n_pad, np.int64)
    import heapq
    heap = [(0, b) for b in range(tot_win)]
    heapq.heapify(heap)
    spill = []
    for n in order_nodes:
        d = deg[n]
        tmp = []
        placed = False
        while heap:
            load, b = heapq.heappop(heap)
            if counts[b] < P and (load + d <= CAP or d == 0):
                pid[n] = b * P + counts[b]
                counts[b] += 1
                loads[b] = load + d
                heapq.heappush(heap, (loads[b], b))
                placed = True
                break
            tmp.append((load, b))
        for it in tmp:
            heapq.heappush(heap, it)
        if not placed:
            spill.append(n)
    for n in spill:  # capacity exceeded somewhere: place least-loaded open bin
        cand = [b for b in range(tot_win) if counts[b] < P]
        b = min(cand, key=lambda bb: loads[bb])
        pid[n] = b * P + counts[b]
        counts[b] += 1
        loads[b] += deg[n]
    M = max(1, int(math.ceil(loads.max() / P)))

    psrc = pid[src]
    pdst = pid[dst]

    order = np.argsort(pdst, kind="stable")
    dsts = pdst[order]
    wbound = np.searchsorted(dsts, np.arange(tot_win + 1) * P)

    # inverse permutation for x: xperm[p] = x[orig node with pid p]
    inv = np.zeros(n_pad, np.int64)
    inv[pid] = np.arange(n_pad)
    xperm = np.zeros((n_pad, IN), np.float32)
    real = inv < n_nodes
    xperm[real] = x[inv[real]]

    cores = []
    for cidx in range(NC_CORES):
        srcg = np.zeros((P, nwin * M), np.int32)
        xe = np.zeros((nwin, M * P, IN), np.float32)
        attrT = np.zeros((nwin, ED, M * P), np.float32)
        attr_rows = np.zeros((nwin, P, M * (ED + 1)), np.float32)
        dstwin = np.full((nwin, M, P), -1.0, np.float32)
        outmap = np.full((nwin, M, P), -1, np.int64)
        for w in range(nwin):
            gw = cidx * nwin + w
            e0, e1 = wbound[gw], wbound[gw + 1]
            cnt = e1 - e0
            if cnt == 0:
                continue
            eids = order[e0:e1]
            j = np.arange(cnt) // P
            p = np.arange(cnt) % P
            srcg[p, w * M + j] = psrc[eids]
            xe[w, j * P + p, :] = x[src[eids]]
            a = edge_attr[eids]
            attrT[w][:, j * P + p] = a.T
            attr_rows[w, p[:, None],
                      (j * (ED + 1))[:, None] + np.arange(ED)[None, :]] = a
            attr_rows[w, p, j * (ED + 1) + ED] = 1.0
            dstwin[w, j, p] = (pdst[eids] - gw * P).astype(np.float32)
            outmap[w, j, p] = eids
        oneh = (dstwin[..., None] == np.arange(P, dtype=np.float32)
                ).astype(np.float32)           # [nwin, M, Pe, Pn]
        S_d = np.ascontiguousarray(
            oneh.transpose(0, 2, 1, 3).reshape(nwin, P, M * P))
        St_d = np.ascontiguousarray(
            oneh.transpose(0, 3, 1, 2).reshape(nwin, P, M * P))
        xeT = np.ascontiguousarray(xe.transpose(0, 2, 1))   # [nwin, IN, M*P]
        cores.append(dict(
            S_d=_bf16(S_d), St_d=_bf16(St_d),
            xeT=_bf16(xeT), attrT=_bf16(attrT), attr_rows=_bf16(attr_rows),
            srcg=srcg, outmap=outmap,
            xpermT=np.ascontiguousarray(
                xperm[cidx * npc:(cidx + 1) * npc].T),       # [IN, npc] f32
        ))
    return cores, n_pad, npc, nwin, M


def build_nc(nwin, M, n_pad, npc, nocoll=False):
    nc = bass.Bass()
    NT = nwin * M
    NG = (M + G - 1) // G
    EW1 = HC + H + ED + 1          # pagg layer1 cols
    EW2 = OUT + 1

    def param(name, shape, dt=F32):
        return nc.declare_dram_parameter(name, list(shape), dt, isOutput=False)

    xT = param("xT", [IN, npc], BF16)
    xeT = param("xeT", [nwin, IN, M * P], BF16)
    S_d = param("S_d", [nwin, P, M * P], BF16)
    St_d = param("St_d", [nwin, P, M * P], BF16)
    attrT = param("attrT", [nwin, ED, M * P], BF16)
    attr_rows = param("attr_rows", [nwin, P, M * (ED + 1)], BF16)
    srcg = param("srcg", [P, NT], I32)
    W1cat = param("W1cat", [IN, 2 * HC], BF16)
    b1cat = param("b1cat", [P, 2 * HC])
    We1 = param("We1", [ED, HC], BF16)
    attS1 = param("attS1", [HC, H], BF16)
    att1bc = param("att1bc", [P, HC])
    gb1 = param("gb1", [P, HC])
    W2cat = param("W2cat", [HC, 2 * OUT], BF16)
    b2cat = param("b2cat", [P, 2 * OUT])
    We2 = param("We2", [ED, OUT], BF16)
    att2col = param("att2col", [OUT, 1], BF16)
    att2bc = param("att2bc", [P, OUT])
    gb2 = param("gb2", [P, OUT])
    WmCat = param("WmCat", [OUT, 2 * OUT], BF16)
    bmv = param("bmv", [P, OUT])
    wm2col = param("wm2col", [OUT, 1], BF16)
    bm2 = param("bm2", [P, 1])
    identb = param("identb", [P, P], BF16)
    identf = param("identf", [P, P])
    outv = nc.declare_dram_parameter("outv", [nwin, M * P], F32, isOutput=True)

    with tile.TileContext(nc) as tc:
        with (
            tc.tile_pool(name="const", bufs=1) as cp,
            tc.tile_pool(name="bulk", bufs=1) as bulk,
            tc.tile_pool(name="win", bufs=3) as wp,
            tc.tile_pool(name="grp", bufs=5) as gp,
            tc.tile_pool(name="tl", bufs=2) as tp,
            tc.tile_pool(name="gat", bufs=16) as gat,
            tc.tile_pool(name="psA", bufs=2, space="PSUM") as psA,
            tc.tile_pool(name="psB", bufs=2, space="PSUM") as psB,
            tc.tile_pool(name="psC", bufs=2, space="PSUM") as psC,
            tc.tile_pool(name="psD", bufs=2, space="PSUM") as psD,
            tc.tile_pool(name="dram", bufs=1, space="DRAM") as dram,
        ):
            # ---- constants
            def ld(ap, shape, dt=F32):
                t = cp.tile(list(shape), dt, tag=f"c{ld.i}")
                ld.i += 1
                nc.sync.dma_start(out=t[:], in_=ap[:])
                return t
            ld.i = 0

            tW1 = ld(W1cat, [IN, 2 * HC], BF16)
            tb1c = ld(b1cat, [P, 2 * HC])
            tWe1 = ld(We1, [ED, HC], BF16)
            tattS1 = ld(attS1, [HC, H], BF16)
            tatt1bc = ld(att1bc, [P, HC])
            tgb1 = ld(gb1, [P, HC])
            tW2 = ld(W2cat, [HC, 2 * OUT], BF16)
            tb2c = ld(b2cat, [P, 2 * OUT])
            tWe2 = ld(We2, [ED, OUT], BF16)
            tatt2c = ld(att2col, [OUT, 1], BF16)
            tatt2bc = ld(att2bc, [P, OUT])
            tgb2 = ld(gb2, [P, OUT])
            tWm = ld(WmCat, [OUT, 2 * OUT], BF16)
            tbmv = ld(bmv, [P, OUT])
            twm2 = ld(wm2col, [OUT, 1], BF16)
            tbm2 = ld(bm2, [P, 1])
            tidb = ld(identb, [P, P], BF16)
            tidf = ld(identf, [P, P])

            txT = bulk.tile([IN, npc], BF16, tag="xT")
            nc.sync.dma_start(out=txT[:], in_=xT[:])
            tsrc = bulk.tile([P, NT], I32, tag="srcg")
            nc.sync.dma_start(out=tsrc[:], in_=srcg[:])

            # node-shard SBUF tables (windows side by side)
            xl_sb = bulk.tile([P, nwin * HC], BF16, tag="xl_sb")
            xr_sb = bulk.tile([P, nwin * HC], BF16, tag="xr_sb")
            xl2_sb = bulk.tile([P, nwin * OUT], BF16, tag="xl2_sb")
            xr2_sb = bulk.tile([P, nwin * OUT], BF16, tag="xr2_sb")
            v_sb = bulk.tile([P, nwin * OUT], BF16, tag="v_sb")
            latT_sb = bulk.tile([ED, nwin * P], BF16, tag="latT_sb")

            # internal DRAM
            xl2_sh = dram.tile([npc, OUT], BF16, tag="xl2_sh")
            xl2_full = dram.tile([n_pad, OUT], BF16, tag="xl2_full")
            u_sh = dram.tile([npc, OUT], BF16, tag="u_sh")
            u_full = dram.tile([n_pad, OUT], BF16, tag="u_full")

            def allgather(shard, full):
                if nocoll:
                    nc.sync.dma_start(out=full[0:npc, :], in_=shard[:])
                else:
                    nc.gpsimd.collective_compute(
                        "AllGather", OP.bypass,
                        replica_groups=[list(range(NC_CORES))],
                        ins=[shard.opt()], outs=[full.opt()],
                    )

            # ================= Phase 0: xl/xr projections into SBUF
            for w in range(nwin):
                pm = psA.tile([P, 2 * HC], F32, tag="psA", space="PSUM")
                nc.tensor.matmul(out=pm[:], lhsT=txT[:, w * P:(w + 1) * P],
                                 rhs=tW1[:], start=True, stop=True)
                nc.scalar.copy(out=xl_sb[:, w * HC:(w + 1) * HC],
                               in_=pm[:, :HC])
                nc.vector.tensor_add(out=xr_sb[:, w * HC:(w + 1) * HC],
                                     in0=pm[:, HC:], in1=tb1c[:, HC:])

            # ================= Layer 1 (no gathers)
            for w in range(nwin):
                txe = wp.tile([IN, M * P], BF16, tag="xe")
                nc.sync.dma_start(out=txe[:], in_=xeT[w])
                tS = wp.tile([P, M * P], BF16, tag="S")
                nc.sync.dma_start(out=tS[:], in_=S_d[w])
                tSt = wp.tile([P, M * P], BF16, tag="St")
                nc.sync.dma_start(out=tSt[:], in_=St_d[w])
                taT = wp.tile([ED, M * P], BF16, tag="aT")
                nc.sync.dma_start(out=taT[:], in_=attrT[w])
                tar = wp.tile([P, M * (ED + 1)], BF16, tag="ar")
                nc.sync.dma_start(out=tar[:], in_=attr_rows[w])
                xlw = xl_sb[:, w * HC:(w + 1) * HC]
                xrw = xr_sb[:, w * HC:(w + 1) * HC]
                pagg = psB.tile([P, EW1], F32, tag="psB", space="PSUM")
                gdef = []
                for g in range(NG):
                    j0 = g * G
                    jn = min(G, M - j0)
                    W = jn * P
                    es = slice(j0 * P, j0 * P + W)
                    mT = psA.tile([HC, G * P], F32, tag="psA", space="PSUM")
                    nc.tensor.matmul(out=mT[:, :W], lhsT=tW1[:, :HC],
                                     rhs=txe[:, es], start=True, stop=False)
                    nc.tensor.matmul(out=mT[:, :W], lhsT=xrw,
                                     rhs=tSt[:, es], start=False, stop=False)
                    nc.tensor.matmul(out=mT[:, :W], lhsT=tWe1[:],
                                     rhs=taT[:, es], start=False, stop=True)
                    gdef.append((j0, jn, W, es, mT))
                glk = []
                for (j0, jn, W, es, mT) in gdef:
                    mlk = gp.tile([HC, G * P], BF16, tag="mlk")
                    nc.scalar.activation(out=mlk[:, :W], in_=mT[:, :W],
                                         func=AF.Prelu, alpha=NEG)
                    glk.append(mlk)
                glg = []
                for (j0, jn, W, es, mT), mlk in zip(gdef, glk):
                    lg = psD.tile([H, G * P], F32, tag="psD", space="PSUM")
                    nc.tensor.matmul(out=lg[:, :W], lhsT=tattS1[:],
                                     rhs=mlk[:, :W], start=True, stop=True)
                    glg.append(lg)
                gex = []
                for (j0, jn, W, es, mT), lg in zip(gdef, glg):
                    exT = gp.tile([H, G * P], BF16, tag="exT")
                    nc.scalar.activation(out=exT[:, :W], in_=lg[:, :W],
                                         func=AF.Exp)
                    gex.append(exT)
                gsb = []
                for (j0, jn, W, es, mT), exT in zip(gdef, gex):
                    exg = psC.tile([P, HC], F32, tag="psC", space="PSUM")
                    for jj in range(jn):
                        nc.tensor.matmul(
                            out=exg[:, jj * H:(jj + 1) * H],
                            lhsT=exT[:, jj * P:(jj + 1) * P],
                            rhs=tidb[0:H, 0:H], start=True, stop=True)
                    exsb = gat.tile([P, G * H], BF16, tag="exsb")
                    nc.scalar.copy(out=exsb[:, :jn * H], in_=exg[:, :jn * H])
                    gsb.append(exsb)
                for (j0, jn, W, es, mT), exsb in zip(gdef, gsb):
                    for j in range(j0, j0 + jn):
                        jj = j - j0
                        tsl = slice(j * P, (j + 1) * P)
                        slp = psC.tile([P, HC], F32, tag="psC", space="PSUM")
                        nc.tensor.matmul(out=slp[:], lhsT=txe[:, tsl],
                                         rhs=tW1[:, :HC], start=True, stop=True)
                        rv = gat.tile([P, HC], BF16, tag="rv")
                        nc.vector.tensor_tensor(
                            out=rv[:].rearrange("p (h c) -> p h c", h=H),
                            in0=slp[:].rearrange("p (h c) -> p h c", h=H),
                            in1=exsb[:, jj * H:(jj + 1) * H]
                                .rearrange("p (h o) -> p h o", o=1)
                                .to_broadcast([P, H, C]),
                            op=OP.mult)
                        nc.tensor.matmul(out=pagg[:, :HC], lhsT=tS[:, tsl],
                                         rhs=rv[:],
                                         start=(j == 0), stop=(j == M - 1))
                        nc.tensor.matmul(out=pagg[:, HC:HC + H],
                                         lhsT=tS[:, tsl],
                                         rhs=exsb[:, jj * H:(jj + 1) * H],
                                         start=(j == 0), stop=(j == M - 1))
                        nc.tensor.matmul(
                            out=pagg[:, HC + H:], lhsT=tS[:, tsl],
                            rhs=tar[:, j * (ED + 1):(j + 1) * (ED + 1)],
                            start=(j == 0), stop=(j == M - 1))
                # ---- window tail
                cnt = tp.tile([P, 1], F32, tag="cnt")
                nc.vector.tensor_scalar(out=cnt[:], in0=pagg[:, EW1 - 1:],
                                        scalar1=1.0, scalar2=None, op0=OP.max)
                rcc = tp.tile([P, 1], F32, tag="rcc")
                nc.vector.reciprocal(out=rcc[:], in_=cnt[:])
                lat = tp.tile([P, ED], F32, tag="lat")
                nc.vector.tensor_scalar(out=lat[:],
                                        in0=pagg[:, HC + H:HC + H + ED],
                                        scalar1=rcc[:, :1], scalar2=None,
                                        op0=OP.mult)
                ptr = psD.tile([ED, P], F32, tag="psD", space="PSUM")
                nc.tensor.matmul(out=ptr[:], lhsT=lat[:], rhs=tidf[:],
                                 start=True, stop=True)
                nc.scalar.copy(out=latT_sb[:, w * P:(w + 1) * P], in_=ptr[:])
                pel = psC.tile([P, HC], F32, tag="psC", space="PSUM")
                nc.tensor.matmul(out=pel[:],
                                 lhsT=latT_sb[:, w * P:(w + 1) * P],
                                 rhs=tWe1[:], start=True, stop=True)
                ml = tp.tile([P, HC], F32, tag="ml")
                nc.vector.tensor_add(out=ml[:], in0=xlw, in1=xrw)
                nc.vector.tensor_add(out=ml[:], in0=ml[:], in1=pel[:])
                mlk2 = tp.tile([P, HC], F32, tag="mlk2")
                nc.scalar.activation(out=mlk2[:], in_=ml[:], func=AF.Prelu,
                                     alpha=NEG)
                nc.vector.tensor_mul(out=mlk2[:], in0=mlk2[:], in1=tatt1bc[:])
                exl = tp.tile([P, H], F32, tag="exl")
                nc.vector.tensor_reduce(
                    out=exl[:], in_=mlk2[:].rearrange("p (h c) -> p h c", h=H),
                    axis=AX.X, op=OP.add)
                nc.scalar.activation(out=exl[:], in_=exl[:], func=AF.Exp)
                den = tp.tile([P, H], F32, tag="den")
                nc.vector.tensor_add(out=den[:], in0=pagg[:, HC:HC + H],
                                     in1=exl[:])
                rec = tp.tile([P, H], F32, tag="rec")
                nc.vector.reciprocal(out=rec[:], in_=den[:])
                hout = tp.tile([P, HC], F32, tag="hout")
                for hh in range(H):
                    sli = slice(hh * C, (hh + 1) * C)
                    nc.scalar.activation(out=hout[:, sli], in_=xlw[:, sli],
                                         func=AF.Copy,
                                         scale=exl[:, hh:hh + 1])
                nc.vector.tensor_add(out=hout[:], in0=hout[:],
                                     in1=pagg[:, :HC])
                for hh in range(H):
                    sli = slice(hh * C, (hh + 1) * C)
                    nc.scalar.activation(out=hout[:, sli], in_=hout[:, sli],
                                         func=AF.Copy,
                                         scale=rec[:, hh:hh + 1])
                nc.vector.tensor_add(out=hout[:], in0=hout[:], in1=tgb1[:])
                # ELU
                tmin = tp.tile([P, HC], F32, tag="tmin")
                nc.vector.tensor_scalar(out=tmin[:], in0=hout[:], scalar1=0.0,
                                        scalar2=None, op0=OP.min)
                nc.scalar.activation(out=tmin[:], in_=tmin[:], func=AF.Exp)
                helu = tp.tile([P, HC], BF16, tag="helu")
                nc.vector.tensor_scalar(out=helu[:], in0=hout[:], scalar1=0.0,
                                        scalar2=-1.0, op0=OP.max, op1=OP.add)
                nc.vector.tensor_add(out=helu[:], in0=helu[:], in1=tmin[:])
                ptr2 = psD.tile([HC, P], F32, tag="psD", space="PSUM")
                nc.tensor.matmul(out=ptr2[:], lhsT=helu[:], rhs=tidb[:],
                                 start=True, stop=True)
                heluT = tp.tile([HC, P], BF16, tag="heluT")
                nc.scalar.copy(out=heluT[:], in_=ptr2[:])
                p2 = psD.tile([P, 2 * OUT], F32, tag="psD", space="PSUM")
                nc.tensor.matmul(out=p2[:], lhsT=heluT[:], rhs=tW2[:],
                                 start=True, stop=True)
                nc.scalar.copy(out=xl2_sb[:, w * OUT:(w + 1) * OUT],
                               in_=p2[:, :OUT])
                nc.sync.dma_start(out=xl2_sh[w * P:(w + 1) * P, :],
                                  in_=xl2_sb[:, w * OUT:(w + 1) * OUT])
                nc.vector.tensor_add(out=xr2_sb[:, w * OUT:(w + 1) * OUT],
                                     in0=p2[:, OUT:], in1=tb2c[:, OUT:])

            allgather(xl2_sh, xl2_full)

            # ================= Layer 2 (gathers xl2 rows)
            for w in range(nwin):
                tS = wp.tile([P, M * P], BF16, tag="S")
                nc.sync.dma_start(out=tS[:], in_=S_d[w])
                tSt = wp.tile([P, M * P], BF16, tag="St")
                nc.sync.dma_start(out=tSt[:], in_=St_d[w])
                taT = wp.tile([ED, M * P], BF16, tag="aT")
                nc.sync.dma_start(out=taT[:], in_=attrT[w])
                xl2w = xl2_sb[:, w * OUT:(w + 1) * OUT]
                xr2w = xr2_sb[:, w * OUT:(w + 1) * OUT]
                pagg = psB.tile([P, EW2], F32, tag="psB", space="PSUM")
                sls = []
                for j in range(M):
                    t = w * M + j
                    sl = gat.tile([P, OUT], BF16, tag="sl2")
                    nc.gpsimd.indirect_dma_start(
                        out=sl[:], out_offset=None, in_=xl2_full[:],
                        in_offset=bass.IndirectOffsetOnAxis(
                            ap=tsrc[:, t:t + 1], axis=0))
                    sls.append(sl)
                gdef = []
                for g in range(NG):
                    j0 = g * G
                    jn = min(G, M - j0)
                    W = jn * P
                    es = slice(j0 * P, j0 * P + W)
                    mT = psA.tile([OUT, G * P], F32, tag="psA", space="PSUM")
                    for j in range(j0, j0 + jn):
                        jj = j - j0
                        nc.tensor.matmul(out=mT[:, jj * P:(jj + 1) * P],
                                         lhsT=sls[j][:], rhs=tidb[:],
                                         start=True, stop=False)
                    nc.tensor.matmul(out=mT[:, :W], lhsT=xr2w,
                                     rhs=tSt[:, es], start=False, stop=False)
                    nc.tensor.matmul(out=mT[:, :W], lhsT=tWe2[:],
                                     rhs=taT[:, es], start=False, stop=True)
                    gdef.append((j0, jn, W, es, mT))
                glk = []
                for (j0, jn, W, es, mT) in gdef:
                    mlk = gp.tile([OUT, G * P], BF16, tag="mlk")
                    nc.scalar.activation(out=mlk[:, :W], in_=mT[:, :W],
                                         func=AF.Prelu, alpha=NEG)
                    glk.append(mlk)
                glg = []
                for (j0, jn, W, es, mT), mlk in zip(gdef, glk):
                    lg = psD.tile([1, G * P], F32, tag="psD", space="PSUM")
                    nc.tensor.matmul(out=lg[:, :W], lhsT=tatt2c[:],
                                     rhs=mlk[:, :W], start=True, stop=True)
                    glg.append(lg)
                gex = []
                for (j0, jn, W, es, mT), lg in zip(gdef, glg):
                    exT = gp.tile([1, G * P], BF16, tag="exT")
                    nc.scalar.activation(out=exT[:, :W], in_=lg[:, :W],
                                         func=AF.Exp)
                    gex.append(exT)
                gsb = []
                for (j0, jn, W, es, mT), exT in zip(gdef, gex):
                    exg = psC.tile([P, HC], F32, tag="psC", space="PSUM")
                    for jj in range(jn):
                        nc.tensor.matmul(
                            out=exg[:, jj:jj + 1],
                            lhsT=exT[:, jj * P:(jj + 1) * P],
                            rhs=tidb[0:1, 0:1], start=True, stop=True)
                    exsb = gat.tile([P, G], BF16, tag="exsb")
                    nc.scalar.copy(out=exsb[:, :jn], in_=exg[:, :jn])
                    gsb.append(exsb)
                for (j0, jn, W, es, mT), exsb in zip(gdef, gsb):
                    for j in range(j0, j0 + jn):
                        jj = j - j0
                        tsl = slice(j * P, (j + 1) * P)
                        rv = gat.tile([P, OUT], BF16, tag="rv")
                        nc.vector.tensor_tensor(
                            out=rv[:], in0=sls[j][:],
                            in1=exsb[:, jj:jj + 1].to_broadcast([P, OUT]),
                            op=OP.mult)
                        nc.tensor.matmul(out=pagg[:, :OUT], lhsT=tS[:, tsl],
                                         rhs=rv[:],
                                         start=(j == 0), stop=(j == M - 1))
                        nc.tensor.matmul(out=pagg[:, OUT:], lhsT=tS[:, tsl],
                                         rhs=exsb[:, jj:jj + 1],
                                         start=(j == 0), stop=(j == M - 1))
                # ---- tail
                pel = psC.tile([P, OUT], F32, tag="psC", space="PSUM")
                nc.tensor.matmul(out=pel[:],
                                 lhsT=latT_sb[:, w * P:(w + 1) * P],
                                 rhs=tWe2[:], start=True, stop=True)
                ml = tp.tile([P, OUT], F32, tag="ml2")
                nc.vector.tensor_add(out=ml[:], in0=xl2w, in1=xr2w)
                nc.vector.tensor_add(out=ml[:], in0=ml[:], in1=pel[:])
                mlk2 = tp.tile([P, OUT], F32, tag="mlk22")
                nc.scalar.activation(out=mlk2[:], in_=ml[:], func=AF.Prelu,
                                     alpha=NEG)
                nc.vector.tensor_mul(out=mlk2[:], in0=mlk2[:], in1=tatt2bc[:])
                exl = tp.tile([P, 1], F32, tag="exl2")
                nc.vector.tensor_reduce(out=exl[:], in_=mlk2[:], axis=AX.X,
                                        op=OP.add)
                nc.scalar.activation(out=exl[:], in_=exl[:], func=AF.Exp)
                den = tp.tile([P, 1], F32, tag="den2")
                nc.vector.tensor_add(out=den[:], in0=pagg[:, OUT:], in1=exl[:])
                rec = tp.tile([P, 1], F32, tag="rec2")
                nc.vector.reciprocal(out=rec[:], in_=den[:])
                hout = tp.tile([P, OUT], F32, tag="hout2")
                nc.scalar.activation(out=hout[:], in_=xl2w, func=AF.Copy,
                                     scale=exl[:, :1])
                nc.vector.tensor_add(out=hout[:], in0=hout[:],
                                     in1=pagg[:, :OUT])
                nc.scalar.activation(out=hout[:], in_=hout[:], func=AF.Copy,
                                     scale=rec[:, :1])
                houtb = tp.tile([P, OUT], BF16, tag="houtb")
                nc.vector.tensor_add(out=houtb[:], in0=hout[:], in1=tgb2[:])
                ptr3 = psD.tile([OUT, P], F32, tag="psD", space="PSUM")
                nc.tensor.matmul(out=ptr3[:], lhsT=houtb[:], rhs=tidb[:],
                                 start=True, stop=True)
                h2T = tp.tile([OUT, P], BF16, tag="h2T")
                nc.scalar.copy(out=h2T[:], in_=ptr3[:])
                p3 = psD.tile([P, 2 * OUT], F32, tag="psD", space="PSUM")
                nc.tensor.matmul(out=p3[:], lhsT=h2T[:], rhs=tWm[:],
                                 start=True, stop=True)
                uo = tp.tile([P, OUT], BF16, tag="uo")
                nc.scalar.copy(out=uo[:], in_=p3[:, :OUT])
                nc.sync.dma_start(out=u_sh[w * P:(w + 1) * P, :], in_=uo[:])
                nc.vector.tensor_add(out=v_sb[:, w * OUT:(w + 1) * OUT],
                                     in0=p3[:, OUT:], in1=tbmv[:])

            allgather(u_sh, u_full)

            # ================= Phase 4: edge MLP scores
            for w in range(nwin):
                tSt = wp.tile([P, M * P], BF16, tag="St")
                nc.sync.dma_start(out=tSt[:], in_=St_d[w])
                vw = v_sb[:, w * OUT:(w + 1) * OUT]
                outsb = gp.tile([1, M * P], F32, tag="outsb")
                uss = []
                for j in range(M):
                    t = w * M + j
                    us = gat.tile([P, OUT], BF16, tag="us")
                    nc.gpsimd.indirect_dma_start(
                        out=us[:], out_offset=None, in_=u_full[:],
                        in_offset=bass.IndirectOffsetOnAxis(
                            ap=tsrc[:, t:t + 1], axis=0))
                    uss.append(us)
                gdef = []
                for g in range(NG):
                    j0 = g * G
                    jn = min(G, M - j0)
                    W = jn * P
                    es = slice(j0 * P, j0 * P + W)
                    qT = psA.tile([OUT, G * P], F32, tag="psA", space="PSUM")
                    for j in range(j0, j0 + jn):
                        jj = j - j0
                        nc.tensor.matmul(out=qT[:, jj * P:(jj + 1) * P],
                                         lhsT=uss[j][:], rhs=tidb[:],
                                         start=True, stop=False)
                    nc.tensor.matmul(out=qT[:, :W], lhsT=vw,
                                     rhs=tSt[:, es], start=False, stop=True)
                    gdef.append((j0, jn, W, qT))
                gz = []
                for (j0, jn, W, qT) in gdef:
                    zT = gp.tile([OUT, G * P], BF16, tag="zT")
                    nc.scalar.activation(out=zT[:, :W], in_=qT[:, :W],
                                         func=AF.Relu)
                    gz.append(zT)
                gsc = []
                for (j0, jn, W, qT), zT in zip(gdef, gz):
                    sc = psD.tile([1, G * P], F32, tag="psD", space="PSUM")
                    nc.tensor.matmul(out=sc[:, :W], lhsT=twm2[:],
                                     rhs=zT[:, :W], start=True, stop=True)
                    gsc.append(sc)
                for (j0, jn, W, qT), sc in zip(gdef, gsc):
                    nc.vector.tensor_scalar(
                        out=outsb[:, j0 * P:j0 * P + W], in0=sc[:, :W],
                        scalar1=tbm2[:1, :1], scalar2=None, op0=OP.add)
                nc.sync.dma_start(out=outv[w], in_=outsb[:])
    return nc


def kernel(x, edge_index, edge_attr,
           Wl1, bl1, Wr1, br1, We1, att1, b1,
           Wl2, bl2, Wr2, br2, We2, att2, b2,
           Wm1, bm1, Wm2, bm2):
    x = np.asarray(x, np.float32)
    edge_index = np.asarray(edge_index, np.int32)
    edge_attr = np.asarray(edge_attr, np.float32)
    N = x.shape[0]
    E = edge_index.shape[1]

    cores, n_pad, npc, nwin, M = host_prep(x, edge_index, edge_attr, N)
    f32 = lambda a: np.asarray(a, np.float32)

    def bc(v, width):
        v = np.asarray(v, np.float32).reshape(-1)
        return np.ascontiguousarray(np.broadcast_to(v[None, :width], (P, width)))

    W1cat = _bf16(np.concatenate([f32(Wl1), f32(Wr1)], axis=1))
    b1cat = bc(np.concatenate([np.zeros(HC, np.float32),
                               f32(bl1) + f32(br1)]), 2 * HC)
    att1f = f32(att1).reshape(H, C)
    attS1 = np.zeros((HC, H), np.float32)
    for hh in range(H):
        attS1[hh * C:(hh + 1) * C, hh] = att1f[hh]
    W2cat = _bf16(np.concatenate([f32(Wl2), f32(Wr2)], axis=1))
    b2cat = bc(np.concatenate([np.zeros(OUT, np.float32),
                               f32(bl2) + f32(br2)]), 2 * OUT)
    Wm1f = f32(Wm1)
    WmCat = _bf16(np.concatenate([Wm1f[:OUT, :], Wm1f[OUT:, :]], axis=1))

    shared = dict(
        W1cat=W1cat, b1cat=b1cat, We1=_bf16(We1), attS1=_bf16(attS1),
        att1bc=bc(att1f.reshape(-1), HC), gb1=bc(f32(b1) + f32(bl1), HC),
        W2cat=W2cat, b2cat=b2cat, We2=_bf16(We2),
        att2col=_bf16(f32(att2).reshape(OUT, 1)),
        att2bc=bc(f32(att2).reshape(-1), OUT),
        gb2=bc(f32(b2) + f32(bl2), OUT),
        WmCat=WmCat, bmv=bc(bm1, OUT),
        wm2col=_bf16(f32(Wm2).reshape(OUT, 1)),
        bm2=bc(bm2, 1),
        identb=_bf16(np.eye(P, dtype=np.float32)),
        identf=np.eye(P, dtype=np.float32),
    )

    in_maps = []
    for cidx in range(NC_CORES):
        cd = cores[cidx]
        m = dict(shared)
        m["xT"] = _bf16(cd["xpermT"])
        m["xeT"] = cd["xeT"]
        m["S_d"] = cd["S_d"]
        m["St_d"] = cd["St_d"]
        m["attrT"] = cd["attrT"]
        m["attr_rows"] = cd["attr_rows"]
        m["srcg"] = cd["srcg"]
        in_maps.append(m)

    nc = build_nc(nwin, M, n_pad, npc)
    res = run_bass_kernel_spmd(nc, in_maps, core_ids=list(range(NC_CORES)),
                               trace=_TRACE[0])
    _LAST[0] = res.exec_time_ns

    out = np.zeros((E, 1), np.float32)
    for cidx in range(NC_CORES):
        ov = np.asarray(res.results[cidx]["outv"], np.float32)  # [nwin, M*P]
        ov = ov.reshape(nwin, M, P)
        om = cores[cidx]["outmap"]
        sel = om >= 0
        out[om[sel], 0] = ov[sel]
    return out


# revision 9
# speedup vs baseline: 1.1428x; 1.0406x over previous
"""GATv2 edge predictor on 8 TRN2 NeuronCores — v2.

Sharding: nodes degree-balanced into 392 windows of 128 (host permutation) so
every window holds <= M*128 edges; edges partitioned by dst window across the
8 cores.  Per-edge messages are computed in TRANSPOSED form ([channels, edges])
so the xr/ea/logit stages are single big bf16 matmuls per 512-edge group.
Layer-1 x[src] is delivered by the host in edge-slot order (no device gather);
layers 2 and the edge-MLP gather bf16 node rows via per-tile indirect DMA.
Segment softmax as in v1: exp without max-subtraction, one-hot scatter matmuls
into PSUM, dense self-loop tail per window.
"""
import math
import numpy as np

import concourse.bass as bass
import concourse.tile as tile
import concourse.mybir as mybir
from concourse.bass_utils import run_bass_kernel_spmd

# ---------------------------------------------------------------- wait patch
# This container's walrus build rejects >1 sync-wait command per instruction.
# Hoist extra waits onto single-wait NoOps on the same engine (engine streams
# execute in order, so gating is equivalent), and split the kernel-tail
# drain's per-proc waits the same way.
import bass_rust
from concourse.vector_clock import ScopedClock
from bass_rust import VectorClock as _RVC

_orig_commit = tile.TileContext._commit_and_lower
_ctr = [0]


def _split_commit(self, inst, original_block, old_bb_map, bb_to_exit_bb):
    si = getattr(inst, "sync_info", None)
    if si is not None:
        waits = list(si.on_wait)
        if len(waits) > 1:
            hoist = [w for w in waits if w.wait_reg is None]
            keep = [w for w in waits if w.wait_reg is not None]
            if not keep:
                keep = [hoist.pop()]
            for w in hoist:
                _ctr[0] += 1
                n = mybir.InstNoOp(name=f"TW-{_ctr[0]}", ins=[], outs=[])
                n.engine = inst.engine
                n.sync_info = bass_rust.SyncInfo(on_wait=[w], on_update=[])
                _orig_commit(self, n, original_block, old_bb_map, bb_to_exit_bb)
            inst.sync_info = bass_rust.SyncInfo(
                on_wait=keep, on_update=list(si.on_update)
            )
    return _orig_commit(self, inst, original_block, old_bb_map, bb_to_exit_bb)


def _patched_drain_and_barrier(self, tick_clock, wait_clock):
    ticks = list(tick_clock.global_clock)
    for i, t in enumerate(ticks):
        if t > 0:
            sub = [t if j == i else 0 for j in range(len(ticks))]
            nop_inst = self.nc.sync.nop(nofuse=True).ins
            wait_clock.add_sem_waits(nop_inst, ScopedClock({None: _RVC(sub)}))
    self.nc.sync.drain()
    self.nc.all_engine_barrier()
    assert self.sems is not None
    popped = self.nc._tile_sem_poison_stack.pop()
    assert popped is self._sem_poison
    self.nc.clear_and_free_semaphores(list(self.sems.allocated().values()))
    self.nc.all_engine_barrier()


tile.TileContext._commit_and_lower = _split_commit
tile.TileContext._drain_and_barrier = _patched_drain_and_barrier
# ------------------------------------------------------------ end wait patch

F32 = mybir.dt.float32
BF16 = mybir.dt.bfloat16
I32 = mybir.dt.int32
AF = mybir.ActivationFunctionType
OP = mybir.AluOpType
AX = mybir.AxisListType

NC_CORES = 8
P = 128
IN = 128
H = 2
C = 64
HC = H * C          # 128
OUT = 64
ED = 16
NEG = 0.2
G = 4               # tiles per group (512 edges)

_TRACE = [False]
_LAST = [None]


def _ceil_to(x, m):
    return ((x + m - 1) // m) * m


def _bf16(x):
    import jax.numpy as jnp
    return np.asarray(jnp.asarray(np.asarray(x, np.float32), jnp.bfloat16))


def host_prep(x, edge_index, edge_attr, n_nodes):
    """Structure prep: degree-balanced node permutation + per-core edge-slot
    arrays.  Returns (cores, n_pad, nodes_per_core, nwin, M)."""
    E = edge_index.shape[1]
    src = edge_index[0].astype(np.int64)
    dst = edge_index[1].astype(np.int64)
    x = np.asarray(x, np.float32)

    n_pad = _ceil_to(n_nodes, NC_CORES * P)          # 50176
    npc = n_pad // NC_CORES                          # 6272
    nwin = npc // P                                  # 49
    tot_win = NC_CORES * nwin                        # 392

    # ---- degree-balanced windowing: permute node ids so each window of 128
    # nodes has total in-degree <= CAP.
    deg = np.bincount(dst, minlength=n_pad).astype(np.int64)
    CAP = _ceil_to(max(1, int(math.ceil(E / tot_win))), P)   # 2048
    order_nodes = np.argsort(-deg, kind="stable")
    loads = np.zeros(tot_win, np.int64)
    counts = np.zeros(tot_win, np.int64)
    pid = np.zeros(n_pad, np.int64)
    import heapq
    heap = [(0, b) for b in range(tot_win)]
    heapq.heapify(heap)
    spill = []
    for n in order_nodes:
        d = deg[n]
        tmp = []
        placed = False
        while heap:
            load, b = heapq.heappop(heap)
            if counts[b] < P and (load + d <= CAP or d == 0):
                pid[n] = b * P + counts[b]
                counts[b] += 1
                loads[b] = load + d
                heapq.heappush(heap, (loads[b], b))
                placed = True
                break
            tmp.append((load, b))
        for it in tmp:
            heapq.heappush(heap, it)
        if not placed:
            spill.append(n)
    for n in spill:  # capacity exceeded somewhere: place least-loaded open bin
        cand = [b for b in range(tot_win) if counts[b] < P]
        b = min(cand, key=lambda bb: loads[bb])
        pid[n] = b * P + counts[b]
        counts[b] += 1
        loads[b] += deg[n]
    M = max(1, int(math.ceil(loads.max() / P)))

    psrc = pid[src]
    pdst = pid[dst]

    order = np.argsort(pdst, kind="stable")
    dsts = pdst[order]
    wbound = np.searchsorted(dsts, np.arange(tot_win + 1) * P)

    # inverse permutation for x: xperm[p] = x[orig node with pid p]
    inv = np.zeros(n_pad, np.int64)
    inv[pid] = np.arange(n_pad)
    xperm = np.zeros((n_pad, IN), np.float32)
    real = inv < n_nodes
    xperm[real] = x[inv[real]]

    cores = []
    for cidx in range(NC_CORES):
        srcg = np.zeros((P, nwin * M), np.int32)
        xe = np.zeros((nwin, M * P, IN), np.float32)
        attrT = np.zeros((nwin, ED, M * P), np.float32)
        attr_rows = np.zeros((nwin, P, M * (ED + 1)), np.float32)
        dstwin = np.full((nwin, M, P), -1.0, np.float32)
        outmap = np.full((nwin, M, P), -1, np.int64)
        for w in range(nwin):
            gw = cidx * nwin + w
            e0, e1 = wbound[gw], wbound[gw + 1]
            cnt = e1 - e0
            if cnt == 0:
                continue
            eids = order[e0:e1]
            j = np.arange(cnt) // P
            p = np.arange(cnt) % P
            srcg[p, w * M + j] = psrc[eids]
            xe[w, j * P + p, :] = x[src[eids]]
            a = edge_attr[eids]
            attrT[w][:, j * P + p] = a.T
            attr_rows[w, p[:, None],
                      (j * (ED + 1))[:, None] + np.arange(ED)[None, :]] = a
            attr_rows[w, p, j * (ED + 1) + ED] = 1.0
            dstwin[w, j, p] = (pdst[eids] - gw * P).astype(np.float32)
            outmap[w, j, p] = eids
        oneh = (dstwin[..., None] == np.arange(P, dtype=np.float32)
                ).astype(np.float32)           # [nwin, M, Pe, Pn]
        S_d = np.ascontiguousarray(
            oneh.transpose(0, 2, 1, 3).reshape(nwin, P, M * P))
        St_d = np.ascontiguousarray(
            oneh.transpose(0, 3, 1, 2).reshape(nwin, P, M * P))
        xeT = np.ascontiguousarray(xe.transpose(0, 2, 1))   # [nwin, IN, M*P]
        cores.append(dict(
            S_d=_bf16(S_d), St_d=_bf16(St_d),
            xeT=_bf16(xeT), attrT=_bf16(attrT), attr_rows=_bf16(attr_rows),
            srcg=srcg, outmap=outmap,
            xpermT=np.ascontiguousarray(
                xperm[cidx * npc:(cidx + 1) * npc].T),       # [IN, npc] f32
        ))
    return cores, n_pad, npc, nwin, M


def build_nc(nwin, M, n_pad, npc, nocoll=False):
    nc = bass.Bass()
    NT = nwin * M
    NG = (M + G - 1) // G
    EW1 = HC + H + ED + 1          # pagg layer1 cols
    EW2 = OUT + 1

    def param(name, shape, dt=F32):
        return nc.declare_dram_parameter(name, list(shape), dt, isOutput=False)

    xT = param("xT", [IN, npc], BF16)
    xeT = param("xeT", [nwin, IN, M * P], BF16)
    S_d = param("S_d", [nwin, P, M * P], BF16)
    St_d = param("St_d", [nwin, P, M * P], BF16)
    attrT = param("attrT", [nwin, ED, M * P], BF16)
    attr_rows = param("attr_rows", [nwin, P, M * (ED + 1)], BF16)
    srcg = param("srcg", [P, NT], I32)
    W1cat = param("W1cat", [IN, 2 * HC], BF16)
    b1cat = param("b1cat", [P, 2 * HC])
    We1 = param("We1", [ED, HC], BF16)
    attS1 = param("attS1", [HC, H], BF16)
    att1bc = param("att1bc", [P, HC])
    gb1 = param("gb1", [P, HC])
    W2cat = param("W2cat", [HC, 2 * OUT], BF16)
    b2cat = param("b2cat", [P, 2 * OUT])
    We2 = param("We2", [ED, OUT], BF16)
    att2col = param("att2col", [OUT, 1], BF16)
    att2bc = param("att2bc", [P, OUT])
    gb2 = param("gb2", [P, OUT])
    WmCat = param("WmCat", [OUT, 2 * OUT], BF16)
    bmv = param("bmv", [P, OUT])
    wm2col = param("wm2col", [OUT, 1], BF16)
    bm2 = param("bm2", [P, 1])
    identb = param("identb", [P, P], BF16)
    identf = param("identf", [P, P])
    outv = nc.declare_dram_parameter("outv", [nwin, M * P], F32, isOutput=True)

    with tile.TileContext(nc) as tc:
        with (
            tc.tile_pool(name="const", bufs=1) as cp,
            tc.tile_pool(name="bulk", bufs=1) as bulk,
            tc.tile_pool(name="win", bufs=3) as wp,
            tc.tile_pool(name="grp", bufs=5) as gp,
            tc.tile_pool(name="outp", bufs=2) as outp,
            tc.tile_pool(name="tl", bufs=2) as tp,
            tc.tile_pool(name="gat", bufs=16) as gat,
            tc.tile_pool(name="psA", bufs=2, space="PSUM") as psA,
            tc.tile_pool(name="psB", bufs=2, space="PSUM") as psB,
            tc.tile_pool(name="psC", bufs=2, space="PSUM") as psC,
            tc.tile_pool(name="psD", bufs=2, space="PSUM") as psD,
            tc.tile_pool(name="dram", bufs=1, space="DRAM") as dram,
        ):
            # ---- constants
            def ld(ap, shape, dt=F32):
                t = cp.tile(list(shape), dt, tag=f"c{ld.i}")
                ld.i += 1
                nc.sync.dma_start(out=t[:], in_=ap[:])
                return t
            ld.i = 0

            tW1 = ld(W1cat, [IN, 2 * HC], BF16)
            tb1c = ld(b1cat, [P, 2 * HC])
            tWe1 = ld(We1, [ED, HC], BF16)
            tattS1 = ld(attS1, [HC, H], BF16)
            tatt1bc = ld(att1bc, [P, HC])
            tgb1 = ld(gb1, [P, HC])
            tW2 = ld(W2cat, [HC, 2 * OUT], BF16)
            tb2c = ld(b2cat, [P, 2 * OUT])
            tWe2 = ld(We2, [ED, OUT], BF16)
            tatt2c = ld(att2col, [OUT, 1], BF16)
            tatt2bc = ld(att2bc, [P, OUT])
            tgb2 = ld(gb2, [P, OUT])
            tWm = ld(WmCat, [OUT, 2 * OUT], BF16)
            tbmv = ld(bmv, [P, OUT])
            twm2 = ld(wm2col, [OUT, 1], BF16)
            tbm2 = ld(bm2, [P, 1])
            tidb = ld(identb, [P, P], BF16)
            tidf = ld(identf, [P, P])

            txT = bulk.tile([IN, npc], BF16, tag="xT")
            nc.sync.dma_start(out=txT[:], in_=xT[:])
            tsrc = bulk.tile([P, NT], I32, tag="srcg")
            nc.sync.dma_start(out=tsrc[:], in_=srcg[:])

            # node-shard SBUF tables (windows side by side)
            xl_sb = bulk.tile([P, nwin * HC], BF16, tag="xl_sb")
            xr_sb = bulk.tile([P, nwin * HC], BF16, tag="xr_sb")
            xl2_sb = bulk.tile([P, nwin * OUT], BF16, tag="xl2_sb")
            xr2_sb = bulk.tile([P, nwin * OUT], BF16, tag="xr2_sb")
            v_sb = bulk.tile([P, nwin * OUT], BF16, tag="v_sb")
            latT_sb = bulk.tile([ED, nwin * P], BF16, tag="latT_sb")

            # internal DRAM
            xl2_sh = dram.tile([npc, OUT], BF16, tag="xl2_sh")
            xl2_full = dram.tile([n_pad, OUT], BF16, tag="xl2_full")
            u_sh = dram.tile([npc, OUT], BF16, tag="u_sh")
            u_full = dram.tile([n_pad, OUT], BF16, tag="u_full")

            def allgather(shard, full):
                if nocoll:
                    nc.sync.dma_start(out=full[0:npc, :], in_=shard[:])
                else:
                    nc.gpsimd.collective_compute(
                        "AllGather", OP.bypass,
                        replica_groups=[list(range(NC_CORES))],
                        ins=[shard.opt()], outs=[full.opt()],
                    )

            # ================= Phase 0: xl/xr projections into SBUF
            for w in range(nwin):
                pm = psA.tile([P, 2 * HC], F32, tag="psA", space="PSUM")
                nc.tensor.matmul(out=pm[:], lhsT=txT[:, w * P:(w + 1) * P],
                                 rhs=tW1[:], start=True, stop=True)
                nc.scalar.copy(out=xl_sb[:, w * HC:(w + 1) * HC],
                               in_=pm[:, :HC])
                nc.vector.tensor_add(out=xr_sb[:, w * HC:(w + 1) * HC],
                                     in0=pm[:, HC:], in1=tb1c[:, HC:])

            # ================= Layer 1 (no gathers), window pairs
            def l1_window(w):
                txe = wp.tile([IN, M * P], BF16, tag="xe")
                nc.sync.dma_start(out=txe[:], in_=xeT[w])
                tS = wp.tile([P, M * P], BF16, tag="S")
                nc.sync.dma_start(out=tS[:], in_=S_d[w])
                tSt = wp.tile([P, M * P], BF16, tag="St")
                nc.sync.dma_start(out=tSt[:], in_=St_d[w])
                taT = wp.tile([ED, M * P], BF16, tag="aT")
                nc.sync.dma_start(out=taT[:], in_=attrT[w])
                tar = wp.tile([P, M * (ED + 1)], BF16, tag="ar")
                nc.sync.dma_start(out=tar[:], in_=attr_rows[w])
                xlw = xl_sb[:, w * HC:(w + 1) * HC]
                xrw = xr_sb[:, w * HC:(w + 1) * HC]
                pagg = psB.tile([P, EW1], F32, tag="psB", space="PSUM")
                yield
                gdef = []
                for g in range(NG):
                    j0 = g * G
                    jn = min(G, M - j0)
                    W = jn * P
                    es = slice(j0 * P, j0 * P + W)
                    mT = psA.tile([HC, G * P], F32, tag="psA", space="PSUM")
                    nc.tensor.matmul(out=mT[:, :W], lhsT=tW1[:, :HC],
                                     rhs=txe[:, es], start=True, stop=False)
                    nc.tensor.matmul(out=mT[:, :W], lhsT=xrw,
                                     rhs=tSt[:, es], start=False, stop=False)
                    nc.tensor.matmul(out=mT[:, :W], lhsT=tWe1[:],
                                     rhs=taT[:, es], start=False, stop=True)
                    gdef.append((j0, jn, W, es, mT))
                    yield
                glk = []
                for (j0, jn, W, es, mT) in gdef:
                    mlk = gp.tile([HC, G * P], BF16, tag="mlk")
                    nc.scalar.activation(out=mlk[:, :W], in_=mT[:, :W],
                                         func=AF.Prelu, alpha=NEG)
                    glk.append(mlk)
                yield
                glg = []
                for (j0, jn, W, es, mT), mlk in zip(gdef, glk):
                    lg = psD.tile([H, G * P], F32, tag="psD", space="PSUM")
                    nc.tensor.matmul(out=lg[:, :W], lhsT=tattS1[:],
                                     rhs=mlk[:, :W], start=True, stop=True)
                    glg.append(lg)
                yield
                gex = []
                for (j0, jn, W, es, mT), lg in zip(gdef, glg):
                    exT = gp.tile([H, G * P], BF16, tag="exT")
                    nc.scalar.activation(out=exT[:, :W], in_=lg[:, :W],
                                         func=AF.Exp)
                    gex.append(exT)
                yield
                gsb = []
                for (j0, jn, W, es, mT), exT in zip(gdef, gex):
                    exg = psC.tile([P, HC], F32, tag="psC", space="PSUM")
                    for jj in range(jn):
                        nc.tensor.matmul(
                            out=exg[:, jj * H:(jj + 1) * H],
                            lhsT=exT[:, jj * P:(jj + 1) * P],
                            rhs=tidb[0:H, 0:H], start=True, stop=True)
                    exsb = gat.tile([P, G * H], BF16, tag="exsb")
                    nc.scalar.copy(out=exsb[:, :jn * H], in_=exg[:, :jn * H])
                    gsb.append(exsb)
                    yield
                for (j0, jn, W, es, mT), exsb in zip(gdef, gsb):
                    for j in range(j0, j0 + jn):
                        jj = j - j0
                        tsl = slice(j * P, (j + 1) * P)
                        slp = psC.tile([P, HC], F32, tag="psC", space="PSUM")
                        nc.tensor.matmul(out=slp[:], lhsT=txe[:, tsl],
                                         rhs=tW1[:, :HC], start=True, stop=True)
                        rv = gat.tile([P, HC], BF16, tag="rv")
                        nc.vector.tensor_tensor(
                            out=rv[:].rearrange("p (h c) -> p h c", h=H),
                            in0=slp[:].rearrange("p (h c) -> p h c", h=H),
                            in1=exsb[:, jj * H:(jj + 1) * H]
                                .rearrange("p (h o) -> p h o", o=1)
                                .to_broadcast([P, H, C]),
                            op=OP.mult)
                        nc.tensor.matmul(out=pagg[:, :HC], lhsT=tS[:, tsl],
                                         rhs=rv[:],
                                         start=(j == 0), stop=(j == M - 1))
                        nc.tensor.matmul(out=pagg[:, HC:HC + H],
                                         lhsT=tS[:, tsl],
                                         rhs=exsb[:, jj * H:(jj + 1) * H],
                                         start=(j == 0), stop=(j == M - 1))
                        nc.tensor.matmul(
                            out=pagg[:, HC + H:], lhsT=tS[:, tsl],
                            rhs=tar[:, j * (ED + 1):(j + 1) * (ED + 1)],
                            start=(j == 0), stop=(j == M - 1))
                    yield
                # ---- window tail
                cnt = tp.tile([P, 1], F32, tag="cnt")
                nc.vector.tensor_scalar(out=cnt[:], in0=pagg[:, EW1 - 1:],
                                        scalar1=1.0, scalar2=None, op0=OP.max)
                rcc = tp.tile([P, 1], F32, tag="rcc")
                nc.vector.reciprocal(out=rcc[:], in_=cnt[:])
                lat = tp.tile([P, ED], F32, tag="lat")
                nc.vector.tensor_scalar(out=lat[:],
                                        in0=pagg[:, HC + H:HC + H + ED],
                                        scalar1=rcc[:, :1], scalar2=None,
                                        op0=OP.mult)
                ptr = psD.tile([ED, P], F32, tag="psD", space="PSUM")
                nc.tensor.matmul(out=ptr[:], lhsT=lat[:], rhs=tidf[:],
                                 start=True, stop=True)
                nc.scalar.copy(out=latT_sb[:, w * P:(w + 1) * P], in_=ptr[:])
                pel = psC.tile([P, HC], F32, tag="psC", space="PSUM")
                nc.tensor.matmul(out=pel[:],
                                 lhsT=latT_sb[:, w * P:(w + 1) * P],
                                 rhs=tWe1[:], start=True, stop=True)
                ml = tp.tile([P, HC], F32, tag="ml")
                nc.vector.tensor_add(out=ml[:], in0=xlw, in1=xrw)
                nc.vector.tensor_add(out=ml[:], in0=ml[:], in1=pel[:])
                yield
                mlk2 = tp.tile([P, HC], F32, tag="mlk2")
                nc.scalar.activation(out=mlk2[:], in_=ml[:], func=AF.Prelu,
                                     alpha=NEG)
                nc.vector.tensor_mul(out=mlk2[:], in0=mlk2[:], in1=tatt1bc[:])
                exl = tp.tile([P, H], F32, tag="exl")
                nc.vector.tensor_reduce(
                    out=exl[:], in_=mlk2[:].rearrange("p (h c) -> p h c", h=H),
                    axis=AX.X, op=OP.add)
                nc.scalar.activation(out=exl[:], in_=exl[:], func=AF.Exp)
                den = tp.tile([P, H], F32, tag="den")
                nc.vector.tensor_add(out=den[:], in0=pagg[:, HC:HC + H],
                                     in1=exl[:])
                rec = tp.tile([P, H], F32, tag="rec")
                nc.vector.reciprocal(out=rec[:], in_=den[:])
                yield
                hout = tp.tile([P, HC], F32, tag="hout")
                for hh in range(H):
                    sli = slice(hh * C, (hh + 1) * C)
                    nc.scalar.activation(out=hout[:, sli], in_=xlw[:, sli],
                                         func=AF.Copy,
                                         scale=exl[:, hh:hh + 1])
                nc.vector.tensor_add(out=hout[:], in0=hout[:],
                                     in1=pagg[:, :HC])
                for hh in range(H):
                    sli = slice(hh * C, (hh + 1) * C)
                    nc.scalar.activation(out=hout[:, sli], in_=hout[:, sli],
                                         func=AF.Copy,
                                         scale=rec[:, hh:hh + 1])
                nc.vector.tensor_add(out=hout[:], in0=hout[:], in1=tgb1[:])
                yield
                # ELU
                tmin = tp.tile([P, HC], F32, tag="tmin")
                nc.vector.tensor_scalar(out=tmin[:], in0=hout[:], scalar1=0.0,
                                        scalar2=None, op0=OP.min)
                nc.scalar.activation(out=tmin[:], in_=tmin[:], func=AF.Exp)
                helu = tp.tile([P, HC], BF16, tag="helu")
                nc.vector.tensor_scalar(out=helu[:], in0=hout[:], scalar1=0.0,
                                        scalar2=-1.0, op0=OP.max, op1=OP.add)
                nc.vector.tensor_add(out=helu[:], in0=helu[:], in1=tmin[:])
                yield
                ptr2 = psD.tile([HC, P], F32, tag="psD", space="PSUM")
                nc.tensor.matmul(out=ptr2[:], lhsT=helu[:], rhs=tidb[:],
                                 start=True, stop=True)
                heluT = tp.tile([HC, P], BF16, tag="heluT")
                nc.scalar.copy(out=heluT[:], in_=ptr2[:])
                p2 = psD.tile([P, 2 * OUT], F32, tag="psD", space="PSUM")
                nc.tensor.matmul(out=p2[:], lhsT=heluT[:], rhs=tW2[:],
                                 start=True, stop=True)
                nc.scalar.copy(out=xl2_sb[:, w * OUT:(w + 1) * OUT],
                               in_=p2[:, :OUT])
                nc.sync.dma_start(out=xl2_sh[w * P:(w + 1) * P, :],
                                  in_=xl2_sb[:, w * OUT:(w + 1) * OUT])
                nc.vector.tensor_add(out=xr2_sb[:, w * OUT:(w + 1) * OUT],
                                     in0=p2[:, OUT:], in1=tb2c[:, OUT:])

            def run_group(gens):
                gens = list(gens)
                while gens:
                    keep = []
                    for gq in gens:
                        try:
                            next(gq)
                            keep.append(gq)
                        except StopIteration:
                            pass
                    gens = keep

            for w0 in range(0, nwin, 1):
                run_group([l1_window(w)
                           for w in range(w0, min(w0 + 1, nwin))])

            allgather(xl2_sh, xl2_full)

            # ================= Layer 2 (gathers xl2 rows), window pairs
            def l2_window(w):
                tS = wp.tile([P, M * P], BF16, tag="S")
                nc.sync.dma_start(out=tS[:], in_=S_d[w])
                tSt = wp.tile([P, M * P], BF16, tag="St")
                nc.sync.dma_start(out=tSt[:], in_=St_d[w])
                taT = wp.tile([ED, M * P], BF16, tag="aT")
                nc.sync.dma_start(out=taT[:], in_=attrT[w])
                xl2w = xl2_sb[:, w * OUT:(w + 1) * OUT]
                xr2w = xr2_sb[:, w * OUT:(w + 1) * OUT]
                pagg = psB.tile([P, EW2], F32, tag="psB", space="PSUM")
                sls = []
                stag = f"sl2{w % 2}"
                for j in range(M):
                    t = w * M + j
                    sl = gat.tile([P, OUT], BF16, tag=stag)
                    nc.gpsimd.indirect_dma_start(
                        out=sl[:], out_offset=None, in_=xl2_full[:],
                        in_offset=bass.IndirectOffsetOnAxis(
                            ap=tsrc[:, t:t + 1], axis=0))
                    sls.append(sl)
                yield
                gdef = []
                for g in range(NG):
                    j0 = g * G
                    jn = min(G, M - j0)
                    W = jn * P
                    es = slice(j0 * P, j0 * P + W)
                    mT = psA.tile([OUT, G * P], F32, tag="psA", space="PSUM")
                    for j in range(j0, j0 + jn):
                        jj = j - j0
                        nc.tensor.matmul(out=mT[:, jj * P:(jj + 1) * P],
                                         lhsT=sls[j][:], rhs=tidb[:],
                                         start=True, stop=False)
                    nc.tensor.matmul(out=mT[:, :W], lhsT=xr2w,
                                     rhs=tSt[:, es], start=False, stop=False)
                    nc.tensor.matmul(out=mT[:, :W], lhsT=tWe2[:],
                                     rhs=taT[:, es], start=False, stop=True)
                    gdef.append((j0, jn, W, es, mT))
                    yield
                glk = []
                for (j0, jn, W, es, mT) in gdef:
                    mlk = gp.tile([OUT, G * P], BF16, tag="mlk")
                    nc.scalar.activation(out=mlk[:, :W], in_=mT[:, :W],
                                         func=AF.Prelu, alpha=NEG)
                    glk.append(mlk)
                yield
                glg = []
                for (j0, jn, W, es, mT), mlk in zip(gdef, glk):
                    lg = psD.tile([1, G * P], F32, tag="psD", space="PSUM")
                    nc.tensor.matmul(out=lg[:, :W], lhsT=tatt2c[:],
                                     rhs=mlk[:, :W], start=True, stop=True)
                    glg.append(lg)
                yield
                gex = []
                for (j0, jn, W, es, mT), lg in zip(gdef, glg):
                    exT = gp.tile([1, G * P], BF16, tag="exT")
                    nc.scalar.activation(out=exT[:, :W], in_=lg[:, :W],
                                         func=AF.Exp)
                    gex.append(exT)
                yield
                gsb = []
                for (j0, jn, W, es, mT), exT in zip(gdef, gex):
                    exg = psC.tile([P, HC], F32, tag="psC", space="PSUM")
                    for jj in range(jn):
                        nc.tensor.matmul(
                            out=exg[:, jj:jj + 1],
                            lhsT=exT[:, jj * P:(jj + 1) * P],
                            rhs=tidb[0:1, 0:1], start=True, stop=True)
                    exsb = gat.tile([P, G], BF16, tag="exsb")
                    nc.scalar.copy(out=exsb[:, :jn], in_=exg[:, :jn])
                    gsb.append(exsb)
                    yield
                for (j0, jn, W, es, mT), exsb in zip(gdef, gsb):
                    for j in range(j0, j0 + jn):
                        jj = j - j0
                        tsl = slice(j * P, (j + 1) * P)
                        rv = gat.tile([P, OUT], BF16, tag="rv")
                        nc.vector.tensor_tensor(
                            out=rv[:], in0=sls[j][:],
                            in1=exsb[:, jj:jj + 1].to_broadcast([P, OUT]),
                            op=OP.mult)
                        nc.tensor.matmul(out=pagg[:, :OUT], lhsT=tS[:, tsl],
                                         rhs=rv[:],
                                         start=(j == 0), stop=(j == M - 1))
                        nc.tensor.matmul(out=pagg[:, OUT:], lhsT=tS[:, tsl],
                                         rhs=exsb[:, jj:jj + 1],
                                         start=(j == 0), stop=(j == M - 1))
                    yield
                # ---- tail
                pel = psC.tile([P, OUT], F32, tag="psC", space="PSUM")
                nc.tensor.matmul(out=pel[:],
                                 lhsT=latT_sb[:, w * P:(w + 1) * P],
                                 rhs=tWe2[:], start=True, stop=True)
                ml = tp.tile([P, OUT], F32, tag="ml2")
                nc.vector.tensor_add(out=ml[:], in0=xl2w, in1=xr2w)
                nc.vector.tensor_add(out=ml[:], in0=ml[:], in1=pel[:])
                mlk2 = tp.tile([P, OUT], F32, tag="mlk22")
                nc.scalar.activation(out=mlk2[:], in_=ml[:], func=AF.Prelu,
                                     alpha=NEG)
                nc.vector.tensor_mul(out=mlk2[:], in0=mlk2[:], in1=tatt2bc[:])
                exl = tp.tile([P, 1], F32, tag="exl2")
                nc.vector.tensor_reduce(out=exl[:], in_=mlk2[:], axis=AX.X,
                                        op=OP.add)
                nc.scalar.activation(out=exl[:], in_=exl[:], func=AF.Exp)
                yield
                den = tp.tile([P, 1], F32, tag="den2")
                nc.vector.tensor_add(out=den[:], in0=pagg[:, OUT:], in1=exl[:])
                rec = tp.tile([P, 1], F32, tag="rec2")
                nc.vector.reciprocal(out=rec[:], in_=den[:])
                hout = tp.tile([P, OUT], F32, tag="hout2")
                nc.scalar.activation(out=hout[:], in_=xl2w, func=AF.Copy,
                                     scale=exl[:, :1])
                nc.vector.tensor_add(out=hout[:], in0=hout[:],
                                     in1=pagg[:, :OUT])
                nc.scalar.activation(out=hout[:], in_=hout[:], func=AF.Copy,
                                     scale=rec[:, :1])
                houtb = tp.tile([P, OUT], BF16, tag="houtb")
                nc.vector.tensor_add(out=houtb[:], in0=hout[:], in1=tgb2[:])
                yield
                ptr3 = psD.tile([OUT, P], F32, tag="psD", space="PSUM")
                nc.tensor.matmul(out=ptr3[:], lhsT=houtb[:], rhs=tidb[:],
                                 start=True, stop=True)
                h2T = tp.tile([OUT, P], BF16, tag="h2T")
                nc.scalar.copy(out=h2T[:], in_=ptr3[:])
                p3 = psD.tile([P, 2 * OUT], F32, tag="psD", space="PSUM")
                nc.tensor.matmul(out=p3[:], lhsT=h2T[:], rhs=tWm[:],
                                 start=True, stop=True)
                uo = tp.tile([P, OUT], BF16, tag="uo")
                nc.scalar.copy(out=uo[:], in_=p3[:, :OUT])
                nc.sync.dma_start(out=u_sh[w * P:(w + 1) * P, :], in_=uo[:])
                nc.vector.tensor_add(out=v_sb[:, w * OUT:(w + 1) * OUT],
                                     in0=p3[:, OUT:], in1=tbmv[:])

            for w0 in range(0, nwin, 1):
                run_group([l2_window(w)
                           for w in range(w0, min(w0 + 1, nwin))])

            allgather(u_sh, u_full)

            # ================= Phase 4: edge MLP scores
            for w in range(nwin):
                tSt = wp.tile([P, M * P], BF16, tag="St")
                nc.sync.dma_start(out=tSt[:], in_=St_d[w])
                vw = v_sb[:, w * OUT:(w + 1) * OUT]
                outsb = gp.tile([1, M * P], F32, tag="outsb")
                uss = []
                for j in range(M):
                    t = w * M + j
                    us = gat.tile([P, OUT], BF16, tag="us")
                    nc.gpsimd.indirect_dma_start(
                        out=us[:], out_offset=None, in_=u_full[:],
                        in_offset=bass.IndirectOffsetOnAxis(
                            ap=tsrc[:, t:t + 1], axis=0))
                    uss.append(us)
                gdef = []
                for g in range(NG):
                    j0 = g * G
                    jn = min(G, M - j0)
                    W = jn * P
                    es = slice(j0 * P, j0 * P + W)
                    qT = psA.tile([OUT, G * P], F32, tag="psA", space="PSUM")
                    for j in range(j0, j0 + jn):
                        jj = j - j0
                        nc.tensor.matmul(out=qT[:, jj * P:(jj + 1) * P],
                                         lhsT=uss[j][:], rhs=tidb[:],
                                         start=True, stop=False)
                    nc.tensor.matmul(out=qT[:, :W], lhsT=vw,
                                     rhs=tSt[:, es], start=False, stop=True)
                    gdef.append((j0, jn, W, qT))
                gz = []
                for (j0, jn, W, qT) in gdef:
                    zT = gp.tile([OUT, G * P], BF16, tag="zT")
                    nc.scalar.activation(out=zT[:, :W], in_=qT[:, :W],
                                         func=AF.Relu)
                    gz.append(zT)
                gsc = []
                for (j0, jn, W, qT), zT in zip(gdef, gz):
                    sc = psD.tile([1, G * P], F32, tag="psD", space="PSUM")
                    nc.tensor.matmul(out=sc[:, :W], lhsT=twm2[:],
                                     rhs=zT[:, :W], start=True, stop=True)
                    gsc.append(sc)
                for (j0, jn, W, qT), sc in zip(gdef, gsc):
                    nc.vector.tensor_scalar(
                        out=outsb[:, j0 * P:j0 * P + W], in0=sc[:, :W],
                        scalar1=tbm2[:1, :1], scalar2=None, op0=OP.add)
                nc.sync.dma_start(out=outv[w], in_=outsb[:])
    return nc


def kernel(x, edge_index, edge_attr,
           Wl1, bl1, Wr1, br1, We1, att1, b1,
           Wl2, bl2, Wr2, br2, We2, att2, b2,
           Wm1, bm1, Wm2, bm2):
    x = np.asarray(x, np.float32)
    edge_index = np.asarray(edge_index, np.int32)
    edge_attr = np.asarray(edge_attr, np.float32)
    N = x.shape[0]
    E = edge_index.shape[1]

    cores, n_pad, npc, nwin, M = host_prep(x, edge_index, edge_attr, N)
    f32 = lambda a: np.asarray(a, np.float32)

    def bc(v, width):
        v = np.asarray(v, np.float32).reshape(-1)
        return np.ascontiguousarray(np.broadcast_to(v[None, :width], (P, width)))

    W1cat = _bf16(np.concatenate([f32(Wl1), f32(Wr1)], axis=1))
    b1cat = bc(np.concatenate([np.zeros(HC, np.float32),
                               f32(bl1) + f32(br1)]), 2 * HC)
    att1f = f32(att1).reshape(H, C)
    attS1 = np.zeros((HC, H), np.float32)
    for hh in range(H):
        attS1[hh * C:(hh + 1) * C, hh] = att1f[hh]
    W2cat = _bf16(np.concatenate([f32(Wl2), f32(Wr2)], axis=1))
    b2cat = bc(np.concatenate([np.zeros(OUT, np.float32),
                               f32(bl2) + f32(br2)]), 2 * OUT)
    Wm1f = f32(Wm1)
    WmCat = _bf16(np.concatenate([Wm1f[:OUT, :], Wm1f[OUT:, :]], axis=1))

    shared = dict(
        W1cat=W1cat, b1cat=b1cat, We1=_bf16(We1), attS1=_bf16(attS1),
        att1bc=bc(att1f.reshape(-1), HC), gb1=bc(f32(b1) + f32(bl1), HC),
        W2cat=W2cat, b2cat=b2cat, We2=_bf16(We2),
        att2col=_bf16(f32(att2).reshape(OUT, 1)),
        att2bc=bc(f32(att2).reshape(-1), OUT),
        gb2=bc(f32(b2) + f32(bl2), OUT),
        WmCat=WmCat, bmv=bc(bm1, OUT),
        wm2col=_bf16(f32(Wm2).reshape(OUT, 1)),
        bm2=bc(bm2, 1),
        identb=_bf16(np.eye(P, dtype=np.float32)),
        identf=np.eye(P, dtype=np.float32),
    )

    in_maps = []
    for cidx in range(NC_CORES):
        cd = cores[cidx]
        m = dict(shared)
        m["xT"] = _bf16(cd["xpermT"])
        m["xeT"] = cd["xeT"]
        m["S_d"] = cd["S_d"]
        m["St_d"] = cd["St_d"]
        m["attrT"] = cd["attrT"]
        m["attr_rows"] = cd["attr_rows"]
        m["srcg"] = cd["srcg"]
        in_maps.append(m)

    nc = build_nc(nwin, M, n_pad, npc)
    res = run_bass_kernel_spmd(nc, in_maps, core_ids=list(range(NC_CORES)),
                               trace=_TRACE[0])
    _LAST[0] = res.exec_time_ns

    out = np.zeros((E, 1), np.float32)
    for cidx in range(NC_CORES):
        ov = np.asarray(res.results[cidx]["outv"], np.float32)  # [nwin, M*P]
        ov = ov.reshape(nwin, M, P)
        om = cores[cidx]["outmap"]
        sel = om >= 0
        out[om[sel], 0] = ov[sel]
    return out


# revision 11
# speedup vs baseline: 1.1822x; 1.0346x over previous
"""GATv2 edge predictor on 8 TRN2 NeuronCores — v2.

Sharding: nodes degree-balanced into 392 windows of 128 (host permutation) so
every window holds <= M*128 edges; edges partitioned by dst window across the
8 cores.  Per-edge messages are computed in TRANSPOSED form ([channels, edges])
so the xr/ea/logit stages are single big bf16 matmuls per 512-edge group.
Layer-1 x[src] is delivered by the host in edge-slot order (no device gather);
layers 2 and the edge-MLP gather bf16 node rows via per-tile indirect DMA.
Segment softmax as in v1: exp without max-subtraction, one-hot scatter matmuls
into PSUM, dense self-loop tail per window.
"""
import math
import numpy as np

import concourse.bass as bass
import concourse.tile as tile
import concourse.mybir as mybir
from concourse.bass_utils import run_bass_kernel_spmd

# ---------------------------------------------------------------- wait patch
# This container's walrus build rejects >1 sync-wait command per instruction.
# Hoist extra waits onto single-wait NoOps on the same engine (engine streams
# execute in order, so gating is equivalent), and split the kernel-tail
# drain's per-proc waits the same way.
import bass_rust
from concourse.vector_clock import ScopedClock
from bass_rust import VectorClock as _RVC

_orig_commit = tile.TileContext._commit_and_lower
_ctr = [0]


def _split_commit(self, inst, original_block, old_bb_map, bb_to_exit_bb):
    si = getattr(inst, "sync_info", None)
    if si is not None:
        waits = list(si.on_wait)
        if len(waits) > 1:
            hoist = [w for w in waits if w.wait_reg is None]
            keep = [w for w in waits if w.wait_reg is not None]
            if not keep:
                keep = [hoist.pop()]
            for w in hoist:
                _ctr[0] += 1
                n = mybir.InstNoOp(name=f"TW-{_ctr[0]}", ins=[], outs=[])
                n.engine = inst.engine
                n.sync_info = bass_rust.SyncInfo(on_wait=[w], on_update=[])
                _orig_commit(self, n, original_block, old_bb_map, bb_to_exit_bb)
            inst.sync_info = bass_rust.SyncInfo(
                on_wait=keep, on_update=list(si.on_update)
            )
    return _orig_commit(self, inst, original_block, old_bb_map, bb_to_exit_bb)


def _patched_drain_and_barrier(self, tick_clock, wait_clock):
    ticks = list(tick_clock.global_clock)
    for i, t in enumerate(ticks):
        if t > 0:
            sub = [t if j == i else 0 for j in range(len(ticks))]
            nop_inst = self.nc.sync.nop(nofuse=True).ins
            wait_clock.add_sem_waits(nop_inst, ScopedClock({None: _RVC(sub)}))
    self.nc.sync.drain()
    self.nc.all_engine_barrier()
    assert self.sems is not None
    popped = self.nc._tile_sem_poison_stack.pop()
    assert popped is self._sem_poison
    self.nc.clear_and_free_semaphores(list(self.sems.allocated().values()))
    self.nc.all_engine_barrier()


tile.TileContext._commit_and_lower = _split_commit
tile.TileContext._drain_and_barrier = _patched_drain_and_barrier
# ------------------------------------------------------------ end wait patch

F32 = mybir.dt.float32
BF16 = mybir.dt.bfloat16
I32 = mybir.dt.int32
AF = mybir.ActivationFunctionType
OP = mybir.AluOpType
AX = mybir.AxisListType

NC_CORES = 8
P = 128
IN = 128
H = 2
C = 64
HC = H * C          # 128
OUT = 64
ED = 16
NEG = 0.2
G = 4               # tiles per group (512 edges)

_TRACE = [False]
_LAST = [None]


def _ceil_to(x, m):
    return ((x + m - 1) // m) * m


def _bf16(x):
    import jax.numpy as jnp
    return np.asarray(jnp.asarray(np.asarray(x, np.float32), jnp.bfloat16))


def host_prep(x, edge_index, edge_attr, n_nodes):
    """Structure prep: degree-balanced node permutation + per-core edge-slot
    arrays.  Returns (cores, n_pad, nodes_per_core, nwin, M)."""
    E = edge_index.shape[1]
    src = edge_index[0].astype(np.int64)
    dst = edge_index[1].astype(np.int64)
    x = np.asarray(x, np.float32)

    n_pad = _ceil_to(n_nodes, NC_CORES * P)          # 50176
    npc = n_pad // NC_CORES                          # 6272
    nwin = npc // P                                  # 49
    tot_win = NC_CORES * nwin                        # 392

    # ---- degree-balanced windowing: permute node ids so each window of 128
    # nodes has total in-degree <= CAP.
    deg = np.bincount(dst, minlength=n_pad).astype(np.int64)
    CAP = _ceil_to(max(1, int(math.ceil(E / tot_win))), P)   # 2048
    order_nodes = np.argsort(-deg, kind="stable")
    loads = np.zeros(tot_win, np.int64)
    counts = np.zeros(tot_win, np.int64)
    pid = np.zeros(n_pad, np.int64)
    import heapq
    heap = [(0, b) for b in range(tot_win)]
    heapq.heapify(heap)
    spill = []
    for n in order_nodes:
        d = deg[n]
        tmp = []
        placed = False
        while heap:
            load, b = heapq.heappop(heap)
            if counts[b] < P and (load + d <= CAP or d == 0):
                pid[n] = b * P + counts[b]
                counts[b] += 1
                loads[b] = load + d
                heapq.heappush(heap, (loads[b], b))
                placed = True
                break
            tmp.append((load, b))
        for it in tmp:
            heapq.heappush(heap, it)
        if not placed:
            spill.append(n)
    for n in spill:  # capacity exceeded somewhere: place least-loaded open bin
        cand = [b for b in range(tot_win) if counts[b] < P]
        b = min(cand, key=lambda bb: loads[bb])
        pid[n] = b * P + counts[b]
        counts[b] += 1
        loads[b] += deg[n]
    M = max(1, int(math.ceil(loads.max() / P)))

    psrc = pid[src]
    pdst = pid[dst]

    order = np.argsort(pdst, kind="stable")
    dsts = pdst[order]
    wbound = np.searchsorted(dsts, np.arange(tot_win + 1) * P)

    # inverse permutation for x: xperm[p] = x[orig node with pid p]
    inv = np.zeros(n_pad, np.int64)
    inv[pid] = np.arange(n_pad)
    xperm = np.zeros((n_pad, IN), np.float32)
    real = inv < n_nodes
    xperm[real] = x[inv[real]]

    cores = []
    for cidx in range(NC_CORES):
        srcg = np.zeros((P, nwin * M), np.int32)
        xe = np.zeros((nwin, M * P, IN), np.float32)
        attrT = np.zeros((nwin, ED, M * P), np.float32)
        attr_rows = np.zeros((nwin, P, M * (ED + 1)), np.float32)
        dstwin = np.full((nwin, M, P), -1.0, np.float32)
        outmap = np.full((nwin, M, P), -1, np.int64)
        for w in range(nwin):
            gw = cidx * nwin + w
            e0, e1 = wbound[gw], wbound[gw + 1]
            cnt = e1 - e0
            if cnt == 0:
                continue
            eids = order[e0:e1]
            j = np.arange(cnt) // P
            p = np.arange(cnt) % P
            srcg[p, w * M + j] = psrc[eids]
            xe[w, j * P + p, :] = x[src[eids]]
            a = edge_attr[eids]
            attrT[w][:, j * P + p] = a.T
            attr_rows[w, p[:, None],
                      (j * (ED + 1))[:, None] + np.arange(ED)[None, :]] = a
            attr_rows[w, p, j * (ED + 1) + ED] = 1.0
            dstwin[w, j, p] = (pdst[eids] - gw * P).astype(np.float32)
            outmap[w, j, p] = eids
        oneh = (dstwin[..., None] == np.arange(P, dtype=np.float32)
                ).astype(np.float32)           # [nwin, M, Pe, Pn]
        S_d = np.ascontiguousarray(
            oneh.transpose(0, 2, 1, 3).reshape(nwin, P, M * P))
        St_d = np.ascontiguousarray(
            oneh.transpose(0, 3, 1, 2).reshape(nwin, P, M * P))
        xeT = np.ascontiguousarray(xe.transpose(0, 2, 1))   # [nwin, IN, M*P]
        cores.append(dict(
            S_d=_bf16(S_d), St_d=_bf16(St_d),
            xeT=_bf16(xeT), attrT=_bf16(attrT), attr_rows=_bf16(attr_rows),
            srcg=srcg, outmap=outmap,
            xpermT=np.ascontiguousarray(
                xperm[cidx * npc:(cidx + 1) * npc].T),       # [IN, npc] f32
        ))
    return cores, n_pad, npc, nwin, M


def build_nc(nwin, M, n_pad, npc, nocoll=False):
    nc = bass.Bass()
    NT = nwin * M
    NG = (M + G - 1) // G
    EW1 = HC + H + ED + 1          # pagg layer1 cols
    EW2 = OUT + 1

    def param(name, shape, dt=F32):
        return nc.declare_dram_parameter(name, list(shape), dt, isOutput=False)

    xT = param("xT", [IN, npc], BF16)
    xeT = param("xeT", [nwin, IN, M * P], BF16)
    S_d = param("S_d", [nwin, P, M * P], BF16)
    St_d = param("St_d", [nwin, P, M * P], BF16)
    attrT = param("attrT", [nwin, ED, M * P], BF16)
    attr_rows = param("attr_rows", [nwin, P, M * (ED + 1)], BF16)
    srcg = param("srcg", [P, NT], I32)
    W1cat = param("W1cat", [IN, 2 * HC], BF16)
    b1cat = param("b1cat", [P, 2 * HC])
    We1 = param("We1", [ED, HC], BF16)
    attS1 = param("attS1", [HC, H], BF16)
    att1bc = param("att1bc", [P, HC])
    gb1 = param("gb1", [P, HC])
    W2cat = param("W2cat", [HC, 2 * OUT], BF16)
    b2cat = param("b2cat", [P, 2 * OUT])
    We2 = param("We2", [ED, OUT], BF16)
    att2col = param("att2col", [OUT, 1], BF16)
    att2bc = param("att2bc", [P, OUT])
    gb2 = param("gb2", [P, OUT])
    WmCat = param("WmCat", [OUT, 2 * OUT], BF16)
    bmv = param("bmv", [P, OUT])
    wm2col = param("wm2col", [OUT, 1], BF16)
    bm2 = param("bm2", [P, 1])
    identb = param("identb", [P, P], BF16)
    identf = param("identf", [P, P])
    outv = nc.declare_dram_parameter("outv", [nwin, M * P], F32, isOutput=True)

    with tile.TileContext(nc) as tc:
        with (
            tc.tile_pool(name="const", bufs=1) as cp,
            tc.tile_pool(name="bulk", bufs=1) as bulk,
            tc.tile_pool(name="win", bufs=3) as wp,
            tc.tile_pool(name="grp", bufs=5) as gp,
            tc.tile_pool(name="outp", bufs=2) as outp,
            tc.tile_pool(name="tl", bufs=2) as tp,
            tc.tile_pool(name="gat", bufs=16) as gat,
            tc.tile_pool(name="psA", bufs=2, space="PSUM") as psA,
            tc.tile_pool(name="psB", bufs=2, space="PSUM") as psB,
            tc.tile_pool(name="psC", bufs=2, space="PSUM") as psC,
            tc.tile_pool(name="psD", bufs=2, space="PSUM") as psD,
            tc.tile_pool(name="dram", bufs=1, space="DRAM") as dram,
        ):
            # ---- constants
            def ld(ap, shape, dt=F32):
                t = cp.tile(list(shape), dt, tag=f"c{ld.i}")
                ld.i += 1
                nc.sync.dma_start(out=t[:], in_=ap[:])
                return t
            ld.i = 0

            tW1 = ld(W1cat, [IN, 2 * HC], BF16)
            tb1c = ld(b1cat, [P, 2 * HC])
            tWe1 = ld(We1, [ED, HC], BF16)
            tattS1 = ld(attS1, [HC, H], BF16)
            tatt1bc = ld(att1bc, [P, HC])
            tgb1 = ld(gb1, [P, HC])
            tW2 = ld(W2cat, [HC, 2 * OUT], BF16)
            tb2c = ld(b2cat, [P, 2 * OUT])
            tWe2 = ld(We2, [ED, OUT], BF16)
            tatt2c = ld(att2col, [OUT, 1], BF16)
            tatt2bc = ld(att2bc, [P, OUT])
            tgb2 = ld(gb2, [P, OUT])
            tWm = ld(WmCat, [OUT, 2 * OUT], BF16)
            tbmv = ld(bmv, [P, OUT])
            twm2 = ld(wm2col, [OUT, 1], BF16)
            tbm2 = ld(bm2, [P, 1])
            tidb = ld(identb, [P, P], BF16)
            tidf = ld(identf, [P, P])

            txT = bulk.tile([IN, npc], BF16, tag="xT")
            nc.sync.dma_start(out=txT[:], in_=xT[:])
            tsrc = bulk.tile([P, NT], I32, tag="srcg")
            nc.sync.dma_start(out=tsrc[:], in_=srcg[:])

            # node-shard SBUF tables (windows side by side)
            xl_sb = bulk.tile([P, nwin * HC], BF16, tag="xl_sb")
            xr_sb = bulk.tile([P, nwin * HC], BF16, tag="xr_sb")
            xl2_sb = bulk.tile([P, nwin * OUT], BF16, tag="xl2_sb")
            xr2_sb = bulk.tile([P, nwin * OUT], BF16, tag="xr2_sb")
            v_sb = bulk.tile([P, nwin * OUT], BF16, tag="v_sb")
            latT_sb = bulk.tile([ED, nwin * P], BF16, tag="latT_sb")

            # internal DRAM
            xl2_sh = dram.tile([npc, OUT], BF16, tag="xl2_sh")
            xl2_full = dram.tile([n_pad, OUT], BF16, tag="xl2_full")
            u_sh = dram.tile([npc, OUT], BF16, tag="u_sh")
            u_full = dram.tile([n_pad, OUT], BF16, tag="u_full")

            def allgather(shard, full):
                if nocoll:
                    nc.sync.dma_start(out=full[0:npc, :], in_=shard[:])
                else:
                    nc.gpsimd.collective_compute(
                        "AllGather", OP.bypass,
                        replica_groups=[list(range(NC_CORES))],
                        ins=[shard.opt()], outs=[full.opt()],
                    )

            # ================= Phase 0: xl/xr projections into SBUF
            for w in range(nwin):
                pm = psA.tile([P, 2 * HC], F32, tag="psA", space="PSUM")
                nc.tensor.matmul(out=pm[:], lhsT=txT[:, w * P:(w + 1) * P],
                                 rhs=tW1[:], start=True, stop=True)
                nc.scalar.copy(out=xl_sb[:, w * HC:(w + 1) * HC],
                               in_=pm[:, :HC])
                nc.vector.tensor_add(out=xr_sb[:, w * HC:(w + 1) * HC],
                                     in0=pm[:, HC:], in1=tb1c[:, HC:])

            # ================= Layer 1 (no gathers), window pairs
            def l1_window(w):
                txe = wp.tile([IN, M * P], BF16, tag="xe")
                nc.sync.dma_start(out=txe[:], in_=xeT[w])
                tS = wp.tile([P, M * P], BF16, tag="S")
                nc.sync.dma_start(out=tS[:], in_=S_d[w])
                tSt = wp.tile([P, M * P], BF16, tag="St")
                nc.sync.dma_start(out=tSt[:], in_=St_d[w])
                taT = wp.tile([ED, M * P], BF16, tag="aT")
                nc.sync.dma_start(out=taT[:], in_=attrT[w])
                tar = wp.tile([P, M * (ED + 1)], BF16, tag="ar")
                nc.sync.dma_start(out=tar[:], in_=attr_rows[w])
                xlw = xl_sb[:, w * HC:(w + 1) * HC]
                xrw = xr_sb[:, w * HC:(w + 1) * HC]
                pagg = psB.tile([P, EW1], F32, tag="psB", space="PSUM")
                yield
                gdef = []
                glk = []
                for g in range(NG):
                    j0 = g * G
                    jn = min(G, M - j0)
                    W = jn * P
                    es = slice(j0 * P, j0 * P + W)
                    mT = psA.tile([HC, G * P], F32, tag="psA", space="PSUM")
                    nc.tensor.matmul(out=mT[:, :W], lhsT=tW1[:, :HC],
                                     rhs=txe[:, es], start=True, stop=False)
                    nc.tensor.matmul(out=mT[:, :W], lhsT=xrw,
                                     rhs=tSt[:, es], start=False, stop=False)
                    nc.tensor.matmul(out=mT[:, :W], lhsT=tWe1[:],
                                     rhs=taT[:, es], start=False, stop=True)
                    gdef.append((j0, jn, W, es, mT))
                    mlk = gp.tile([HC, G * P], BF16, tag="mlk")
                    nc.scalar.activation(out=mlk[:, :W], in_=mT[:, :W],
                                         func=AF.Prelu, alpha=NEG)
                    glk.append(mlk)
                    yield
                gex = []
                for (j0, jn, W, es, mT), mlk in zip(gdef, glk):
                    lg = psD.tile([H, G * P], F32, tag="psD", space="PSUM")
                    nc.tensor.matmul(out=lg[:, :W], lhsT=tattS1[:],
                                     rhs=mlk[:, :W], start=True, stop=True)
                    exT = gp.tile([H, G * P], BF16, tag="exT")
                    nc.scalar.activation(out=exT[:, :W], in_=lg[:, :W],
                                         func=AF.Exp)
                    gex.append(exT)
                    yield
                gsb = []
                for (j0, jn, W, es, mT), exT in zip(gdef, gex):
                    exg = psC.tile([P, HC], F32, tag="psC", space="PSUM")
                    for jj in range(jn):
                        nc.tensor.matmul(
                            out=exg[:, jj * H:(jj + 1) * H],
                            lhsT=exT[:, jj * P:(jj + 1) * P],
                            rhs=tidb[0:H, 0:H], start=True, stop=True)
                    exsb = gat.tile([P, G * H], BF16, tag="exsb")
                    nc.scalar.copy(out=exsb[:, :jn * H], in_=exg[:, :jn * H])
                    gsb.append(exsb)
                    yield
                for (j0, jn, W, es, mT), exsb in zip(gdef, gsb):
                    for j in range(j0, j0 + jn):
                        jj = j - j0
                        tsl = slice(j * P, (j + 1) * P)
                        slp = psC.tile([P, HC], F32, tag="psC", space="PSUM")
                        nc.tensor.matmul(out=slp[:], lhsT=txe[:, tsl],
                                         rhs=tW1[:, :HC], start=True, stop=True)
                        rv = gat.tile([P, HC], BF16, tag="rv")
                        nc.vector.tensor_tensor(
                            out=rv[:].rearrange("p (h c) -> p h c", h=H),
                            in0=slp[:].rearrange("p (h c) -> p h c", h=H),
                            in1=exsb[:, jj * H:(jj + 1) * H]
                                .rearrange("p (h o) -> p h o", o=1)
                                .to_broadcast([P, H, C]),
                            op=OP.mult)
                        nc.tensor.matmul(out=pagg[:, :HC], lhsT=tS[:, tsl],
                                         rhs=rv[:],
                                         start=(j == 0), stop=(j == M - 1))
                        nc.tensor.matmul(out=pagg[:, HC:HC + H],
                                         lhsT=tS[:, tsl],
                                         rhs=exsb[:, jj * H:(jj + 1) * H],
                                         start=(j == 0), stop=(j == M - 1))
                        nc.tensor.matmul(
                            out=pagg[:, HC + H:], lhsT=tS[:, tsl],
                            rhs=tar[:, j * (ED + 1):(j + 1) * (ED + 1)],
                            start=(j == 0), stop=(j == M - 1))
                    yield
                # ---- window tail
                cnt = tp.tile([P, 1], F32, tag="cnt")
                nc.vector.tensor_scalar(out=cnt[:], in0=pagg[:, EW1 - 1:],
                                        scalar1=1.0, scalar2=None, op0=OP.max)
                rcc = tp.tile([P, 1], F32, tag="rcc")
                nc.vector.reciprocal(out=rcc[:], in_=cnt[:])
                lat = tp.tile([P, ED], F32, tag="lat")
                nc.vector.tensor_scalar(out=lat[:],
                                        in0=pagg[:, HC + H:HC + H + ED],
                                        scalar1=rcc[:, :1], scalar2=None,
                                        op0=OP.mult)
                ptr = psD.tile([ED, P], F32, tag="psD", space="PSUM")
                nc.tensor.matmul(out=ptr[:], lhsT=lat[:], rhs=tidf[:],
                                 start=True, stop=True)
                nc.scalar.copy(out=latT_sb[:, w * P:(w + 1) * P], in_=ptr[:])
                pel = psC.tile([P, HC], F32, tag="psC", space="PSUM")
                nc.tensor.matmul(out=pel[:],
                                 lhsT=latT_sb[:, w * P:(w + 1) * P],
                                 rhs=tWe1[:], start=True, stop=True)
                ml = tp.tile([P, HC], F32, tag="ml")
                nc.vector.tensor_add(out=ml[:], in0=xlw, in1=xrw)
                nc.vector.tensor_add(out=ml[:], in0=ml[:], in1=pel[:])
                yield
                mlk2 = tp.tile([P, HC], F32, tag="mlk2")
                nc.scalar.activation(out=mlk2[:], in_=ml[:], func=AF.Prelu,
                                     alpha=NEG)
                nc.vector.tensor_mul(out=mlk2[:], in0=mlk2[:], in1=tatt1bc[:])
                exl = tp.tile([P, H], F32, tag="exl")
                nc.vector.tensor_reduce(
                    out=exl[:], in_=mlk2[:].rearrange("p (h c) -> p h c", h=H),
                    axis=AX.X, op=OP.add)
                nc.scalar.activation(out=exl[:], in_=exl[:], func=AF.Exp)
                den = tp.tile([P, H], F32, tag="den")
                nc.vector.tensor_add(out=den[:], in0=pagg[:, HC:HC + H],
                                     in1=exl[:])
                rec = tp.tile([P, H], F32, tag="rec")
                nc.vector.reciprocal(out=rec[:], in_=den[:])
                yield
                hout = tp.tile([P, HC], F32, tag="hout")
                for hh in range(H):
                    sli = slice(hh * C, (hh + 1) * C)
                    nc.scalar.activation(out=hout[:, sli], in_=xlw[:, sli],
                                         func=AF.Copy,
                                         scale=exl[:, hh:hh + 1])
                nc.vector.tensor_add(out=hout[:], in0=hout[:],
                                     in1=pagg[:, :HC])
                for hh in range(H):
                    sli = slice(hh * C, (hh + 1) * C)
                    nc.scalar.activation(out=hout[:, sli], in_=hout[:, sli],
                                         func=AF.Copy,
                                         scale=rec[:, hh:hh + 1])
                nc.vector.tensor_add(out=hout[:], in0=hout[:], in1=tgb1[:])
                yield
                # ELU
                tmin = tp.tile([P, HC], F32, tag="tmin")
                nc.vector.tensor_scalar(out=tmin[:], in0=hout[:], scalar1=0.0,
                                        scalar2=None, op0=OP.min)
                nc.scalar.activation(out=tmin[:], in_=tmin[:], func=AF.Exp)
                helu = tp.tile([P, HC], BF16, tag="helu")
                nc.vector.tensor_scalar(out=helu[:], in0=hout[:], scalar1=0.0,
                                        scalar2=-1.0, op0=OP.max, op1=OP.add)
                nc.vector.tensor_add(out=helu[:], in0=helu[:], in1=tmin[:])
                yield
                ptr2 = psD.tile([HC, P], F32, tag="psD", space="PSUM")
                nc.tensor.matmul(out=ptr2[:], lhsT=helu[:], rhs=tidb[:],
                                 start=True, stop=True)
                heluT = tp.tile([HC, P], BF16, tag="heluT")
                nc.scalar.copy(out=heluT[:], in_=ptr2[:])
                p2 = psD.tile([P, 2 * OUT], F32, tag="psD", space="PSUM")
                nc.tensor.matmul(out=p2[:], lhsT=heluT[:], rhs=tW2[:],
                                 start=True, stop=True)
                nc.scalar.copy(out=xl2_sb[:, w * OUT:(w + 1) * OUT],
                               in_=p2[:, :OUT])
                nc.sync.dma_start(out=xl2_sh[w * P:(w + 1) * P, :],
                                  in_=xl2_sb[:, w * OUT:(w + 1) * OUT])
                nc.vector.tensor_add(out=xr2_sb[:, w * OUT:(w + 1) * OUT],
                                     in0=p2[:, OUT:], in1=tb2c[:, OUT:])

            def run_group(gens):
                gens = list(gens)
                while gens:
                    keep = []
                    for gq in gens:
                        try:
                            next(gq)
                            keep.append(gq)
                        except StopIteration:
                            pass
                    gens = keep

            for w0 in range(0, nwin, 2):
                run_group([l1_window(w)
                           for w in range(w0, min(w0 + 2, nwin))])

            allgather(xl2_sh, xl2_full)

            # ================= Layer 2 (gathers xl2 rows), window pairs
            def l2_window(w):
                tS = wp.tile([P, M * P], BF16, tag="S")
                nc.sync.dma_start(out=tS[:], in_=S_d[w])
                tSt = wp.tile([P, M * P], BF16, tag="St")
                nc.sync.dma_start(out=tSt[:], in_=St_d[w])
                taT = wp.tile([ED, M * P], BF16, tag="aT")
                nc.sync.dma_start(out=taT[:], in_=attrT[w])
                xl2w = xl2_sb[:, w * OUT:(w + 1) * OUT]
                xr2w = xr2_sb[:, w * OUT:(w + 1) * OUT]
                pagg = psB.tile([P, EW2], F32, tag="psB", space="PSUM")
                sls = []
                stag = f"sl2{w % 2}"
                for j in range(M):
                    t = w * M + j
                    sl = gat.tile([P, OUT], BF16, tag=stag)
                    nc.gpsimd.indirect_dma_start(
                        out=sl[:], out_offset=None, in_=xl2_full[:],
                        in_offset=bass.IndirectOffsetOnAxis(
                            ap=tsrc[:, t:t + 1], axis=0))
                    sls.append(sl)
                yield
                gdef = []
                glk = []
                for g in range(NG):
                    j0 = g * G
                    jn = min(G, M - j0)
                    W = jn * P
                    es = slice(j0 * P, j0 * P + W)
                    mT = psA.tile([OUT, G * P], F32, tag="psA", space="PSUM")
                    for j in range(j0, j0 + jn):
                        jj = j - j0
                        nc.tensor.matmul(out=mT[:, jj * P:(jj + 1) * P],
                                         lhsT=sls[j][:], rhs=tidb[:],
                                         start=True, stop=False)
                    nc.tensor.matmul(out=mT[:, :W], lhsT=xr2w,
                                     rhs=tSt[:, es], start=False, stop=False)
                    nc.tensor.matmul(out=mT[:, :W], lhsT=tWe2[:],
                                     rhs=taT[:, es], start=False, stop=True)
                    gdef.append((j0, jn, W, es, mT))
                    mlk = gp.tile([OUT, G * P], BF16, tag="mlk")
                    nc.scalar.activation(out=mlk[:, :W], in_=mT[:, :W],
                                         func=AF.Prelu, alpha=NEG)
                    glk.append(mlk)
                    yield
                gex = []
                for (j0, jn, W, es, mT), mlk in zip(gdef, glk):
                    lg = psD.tile([1, G * P], F32, tag="psD", space="PSUM")
                    nc.tensor.matmul(out=lg[:, :W], lhsT=tatt2c[:],
                                     rhs=mlk[:, :W], start=True, stop=True)
                    exT = gp.tile([1, G * P], BF16, tag="exT")
                    nc.scalar.activation(out=exT[:, :W], in_=lg[:, :W],
                                         func=AF.Exp)
                    gex.append(exT)
                    yield
                gsb = []
                for (j0, jn, W, es, mT), exT in zip(gdef, gex):
                    exg = psC.tile([P, HC], F32, tag="psC", space="PSUM")
                    for jj in range(jn):
                        nc.tensor.matmul(
                            out=exg[:, jj:jj + 1],
                            lhsT=exT[:, jj * P:(jj + 1) * P],
                            rhs=tidb[0:1, 0:1], start=True, stop=True)
                    exsb = gat.tile([P, G], BF16, tag="exsb")
                    nc.scalar.copy(out=exsb[:, :jn], in_=exg[:, :jn])
                    gsb.append(exsb)
                    yield
                for (j0, jn, W, es, mT), exsb in zip(gdef, gsb):
                    for j in range(j0, j0 + jn):
                        jj = j - j0
                        tsl = slice(j * P, (j + 1) * P)
                        rv = gat.tile([P, OUT], BF16, tag="rv")
                        nc.vector.tensor_tensor(
                            out=rv[:], in0=sls[j][:],
                            in1=exsb[:, jj:jj + 1].to_broadcast([P, OUT]),
                            op=OP.mult)
                        nc.tensor.matmul(out=pagg[:, :OUT], lhsT=tS[:, tsl],
                                         rhs=rv[:],
                                         start=(j == 0), stop=(j == M - 1))
                        nc.tensor.matmul(out=pagg[:, OUT:], lhsT=tS[:, tsl],
                                         rhs=exsb[:, jj:jj + 1],
                                         start=(j == 0), stop=(j == M - 1))
                    yield
                # ---- tail
                pel = psC.tile([P, OUT], F32, tag="psC", space="PSUM")
                nc.tensor.matmul(out=pel[:],
                                 lhsT=latT_sb[:, w * P:(w + 1) * P],
                                 rhs=tWe2[:], start=True, stop=True)
                ml = tp.tile([P, OUT], F32, tag="ml2")
                nc.vector.tensor_add(out=ml[:], in0=xl2w, in1=xr2w)
                nc.vector.tensor_add(out=ml[:], in0=ml[:], in1=pel[:])
                mlk2 = tp.tile([P, OUT], F32, tag="mlk22")
                nc.scalar.activation(out=mlk2[:], in_=ml[:], func=AF.Prelu,
                                     alpha=NEG)
                nc.vector.tensor_mul(out=mlk2[:], in0=mlk2[:], in1=tatt2bc[:])
                exl = tp.tile([P, 1], F32, tag="exl2")
                nc.vector.tensor_reduce(out=exl[:], in_=mlk2[:], axis=AX.X,
                                        op=OP.add)
                nc.scalar.activation(out=exl[:], in_=exl[:], func=AF.Exp)
                yield
                den = tp.tile([P, 1], F32, tag="den2")
                nc.vector.tensor_add(out=den[:], in0=pagg[:, OUT:], in1=exl[:])
                rec = tp.tile([P, 1], F32, tag="rec2")
                nc.vector.reciprocal(out=rec[:], in_=den[:])
                hout = tp.tile([P, OUT], F32, tag="hout2")
                nc.scalar.activation(out=hout[:], in_=xl2w, func=AF.Copy,
                                     scale=exl[:, :1])
                nc.vector.tensor_add(out=hout[:], in0=hout[:],
                                     in1=pagg[:, :OUT])
                nc.scalar.activation(out=hout[:], in_=hout[:], func=AF.Copy,
                                     scale=rec[:, :1])
                houtb = tp.tile([P, OUT], BF16, tag="houtb")
                nc.vector.tensor_add(out=houtb[:], in0=hout[:], in1=tgb2[:])
                yield
                ptr3 = psD.tile([OUT, P], F32, tag="psD", space="PSUM")
                nc.tensor.matmul(out=ptr3[:], lhsT=houtb[:], rhs=tidb[:],
                                 start=True, stop=True)
                h2T = tp.tile([OUT, P], BF16, tag="h2T")
                nc.scalar.copy(out=h2T[:], in_=ptr3[:])
                p3 = psD.tile([P, 2 * OUT], F32, tag="psD", space="PSUM")
                nc.tensor.matmul(out=p3[:], lhsT=h2T[:], rhs=tWm[:],
                                 start=True, stop=True)
                uo = tp.tile([P, OUT], BF16, tag="uo")
                nc.scalar.copy(out=uo[:], in_=p3[:, :OUT])
                nc.sync.dma_start(out=u_sh[w * P:(w + 1) * P, :], in_=uo[:])
                nc.vector.tensor_add(out=v_sb[:, w * OUT:(w + 1) * OUT],
                                     in0=p3[:, OUT:], in1=tbmv[:])

            for w0 in range(0, nwin, 2):
                run_group([l2_window(w)
                           for w in range(w0, min(w0 + 2, nwin))])

            allgather(u_sh, u_full)

            # ================= Phase 4: edge MLP scores
            for w in range(nwin):
                tSt = wp.tile([P, M * P], BF16, tag="St")
                nc.sync.dma_start(out=tSt[:], in_=St_d[w])
                vw = v_sb[:, w * OUT:(w + 1) * OUT]
                outsb = gp.tile([1, M * P], F32, tag="outsb")
                uss = []
                for j in range(M):
                    t = w * M + j
                    us = gat.tile([P, OUT], BF16, tag="us")
                    nc.gpsimd.indirect_dma_start(
                        out=us[:], out_offset=None, in_=u_full[:],
                        in_offset=bass.IndirectOffsetOnAxis(
                            ap=tsrc[:, t:t + 1], axis=0))
                    uss.append(us)
                gdef = []
                for g in range(NG):
                    j0 = g * G
                    jn = min(G, M - j0)
                    W = jn * P
                    es = slice(j0 * P, j0 * P + W)
                    qT = psA.tile([OUT, G * P], F32, tag="psA", space="PSUM")
                    for j in range(j0, j0 + jn):
                        jj = j - j0
                        nc.tensor.matmul(out=qT[:, jj * P:(jj + 1) * P],
                                         lhsT=uss[j][:], rhs=tidb[:],
                                         start=True, stop=False)
                    nc.tensor.matmul(out=qT[:, :W], lhsT=vw,
                                     rhs=tSt[:, es], start=False, stop=True)
                    gdef.append((j0, jn, W, qT))
                gz = []
                for (j0, jn, W, qT) in gdef:
                    zT = gp.tile([OUT, G * P], BF16, tag="zT")
                    nc.scalar.activation(out=zT[:, :W], in_=qT[:, :W],
                                         func=AF.Relu)
                    gz.append(zT)
                gsc = []
                for (j0, jn, W, qT), zT in zip(gdef, gz):
                    sc = psD.tile([1, G * P], F32, tag="psD", space="PSUM")
                    nc.tensor.matmul(out=sc[:, :W], lhsT=twm2[:],
                                     rhs=zT[:, :W], start=True, stop=True)
                    gsc.append(sc)
                for (j0, jn, W, qT), sc in zip(gdef, gsc):
                    nc.vector.tensor_scalar(
                        out=outsb[:, j0 * P:j0 * P + W], in0=sc[:, :W],
                        scalar1=tbm2[:1, :1], scalar2=None, op0=OP.add)
                nc.sync.dma_start(out=outv[w], in_=outsb[:])
    return nc


def kernel(x, edge_index, edge_attr,
           Wl1, bl1, Wr1, br1, We1, att1, b1,
           Wl2, bl2, Wr2, br2, We2, att2, b2,
           Wm1, bm1, Wm2, bm2):
    x = np.asarray(x, np.float32)
    edge_index = np.asarray(edge_index, np.int32)
    edge_attr = np.asarray(edge_attr, np.float32)
    N = x.shape[0]
    E = edge_index.shape[1]

    cores, n_pad, npc, nwin, M = host_prep(x, edge_index, edge_attr, N)
    f32 = lambda a: np.asarray(a, np.float32)

    def bc(v, width):
        v = np.asarray(v, np.float32).reshape(-1)
        return np.ascontiguousarray(np.broadcast_to(v[None, :width], (P, width)))

    W1cat = _bf16(np.concatenate([f32(Wl1), f32(Wr1)], axis=1))
    b1cat = bc(np.concatenate([np.zeros(HC, np.float32),
                               f32(bl1) + f32(br1)]), 2 * HC)
    att1f = f32(att1).reshape(H, C)
    attS1 = np.zeros((HC, H), np.float32)
    for hh in range(H):
        attS1[hh * C:(hh + 1) * C, hh] = att1f[hh]
    W2cat = _bf16(np.concatenate([f32(Wl2), f32(Wr2)], axis=1))
    b2cat = bc(np.concatenate([np.zeros(OUT, np.float32),
                               f32(bl2) + f32(br2)]), 2 * OUT)
    Wm1f = f32(Wm1)
    WmCat = _bf16(np.concatenate([Wm1f[:OUT, :], Wm1f[OUT:, :]], axis=1))

    shared = dict(
        W1cat=W1cat, b1cat=b1cat, We1=_bf16(We1), attS1=_bf16(attS1),
        att1bc=bc(att1f.reshape(-1), HC), gb1=bc(f32(b1) + f32(bl1), HC),
        W2cat=W2cat, b2cat=b2cat, We2=_bf16(We2),
        att2col=_bf16(f32(att2).reshape(OUT, 1)),
        att2bc=bc(f32(att2).reshape(-1), OUT),
        gb2=bc(f32(b2) + f32(bl2), OUT),
        WmCat=WmCat, bmv=bc(bm1, OUT),
        wm2col=_bf16(f32(Wm2).reshape(OUT, 1)),
        bm2=bc(bm2, 1),
        identb=_bf16(np.eye(P, dtype=np.float32)),
        identf=np.eye(P, dtype=np.float32),
    )

    in_maps = []
    for cidx in range(NC_CORES):
        cd = cores[cidx]
        m = dict(shared)
        m["xT"] = _bf16(cd["xpermT"])
        m["xeT"] = cd["xeT"]
        m["S_d"] = cd["S_d"]
        m["St_d"] = cd["St_d"]
        m["attrT"] = cd["attrT"]
        m["attr_rows"] = cd["attr_rows"]
        m["srcg"] = cd["srcg"]
        in_maps.append(m)

    nc = build_nc(nwin, M, n_pad, npc)
    res = run_bass_kernel_spmd(nc, in_maps, core_ids=list(range(NC_CORES)),
                               trace=_TRACE[0])
    _LAST[0] = res.exec_time_ns

    out = np.zeros((E, 1), np.float32)
    for cidx in range(NC_CORES):
        ov = np.asarray(res.results[cidx]["outv"], np.float32)  # [nwin, M*P]
        ov = ov.reshape(nwin, M, P)
        om = cores[cidx]["outmap"]
        sel = om >= 0
        out[om[sel], 0] = ov[sel]
    return out
